# revision 6
# baseline (speedup 1.0000x reference)
"""TRN2 Bass kernel for nn_Decode (Bahdanau-attention GRU decode step + vocab head).

Sharding across 8 NeuronCores (SPMD, one program, per-core data):
  - Attention: data-parallel over batch (16 rows/core); small attention weights
    replicated (shipped bf16).
  - GRU + fc/BN: tensor-parallel over the hidden dim (128 of 1024 per core),
    stitched with AllGather collectives.
  - fc2 (vocab head) + embedding gather: tensor-parallel over vocab
    (6283 of 50257 rows per core).
  - Host does layout prep only: weight transposes/casts, embedding row gather,
    BN constant folding, output concatenation.

Numerics: bf16 operands for attention/GRU matmuls (≈1e-4 relative error on
outputs), fp32 for the fc/BN/fc2 chain unless FC2_BF16 is set.
"""

import numpy as np

N_CORES = 8
B, L, EMB, U, V = 128, 64, 512, 1024, 50257
BC = B // N_CORES  # 16 batch rows per core
US = U // N_CORES  # 128 hidden slice per core (GRU/fc TP)
VS = -(-V // N_CORES)  # 6283 vocab rows per core
BN_EPS = 1e-3

FC2_BF16 = False  # fc2 weights/matmul in bf16 (halves the dominant DMA)

ROWS = BC * L  # 1024 attention rows per core
RT = ROWS // 128  # 8 row chunks
EC = EMB // 128  # 4 embedding chunks
UC = U // 128  # 8 hidden chunks
NVC = -(-VS // 512)  # 13 vocab column chunks (last is 139 wide)

_CACHE = {}


def _build_bass():
    import concourse.tile as tile
    from concourse import bacc, masks, mybir

    f32 = mybir.dt.float32
    bf16 = mybir.dt.bfloat16
    AF = mybir.ActivationFunctionType
    OP = mybir.AluOpType
    fc2_dt = bf16 if FC2_BF16 else f32

    nc = bacc.Bacc(None, target_bir_lowering=False, num_devices=N_CORES)

    din = lambda n, s, d: nc.dram_tensor(n, s, d, kind="ExternalInput")
    feat16_d = din("feat16", [ROWS, EMB], bf16)  # this core's features, natural
    featT16_d = din("featT16", [EMB, ROWS], bf16)  # transposed
    uaT16_d = din("uaT16", [EMB, U], bf16)  # Uattn_w.T (replicated)
    waT16_d = din("waT16", [U, U], bf16)  # Wattn_w.T (replicated)
    hT16_d = din("hT16", [U, B], bf16)  # full h.T (replicated)
    hTbc16_d = din("hTbc16", [U, BC], bf16)  # this core's batch cols of h.T
    hU_d = din("hU", [B, US], f32)  # this core's hidden-slice of h, natural
    sbias_d = din("sbias", [128, UC], f32)  # (Uattn_b+Wattn_b) col-chunked
    vT16_d = din("vT16", [128, UC], bf16)  # Vattn_w col-chunked
    eT16_d = din("eT16", [EMB, B], bf16)  # gathered embeddings.T (replicated)
    wrzih_d = din("wrzih", [U, 2 * US], bf16)  # W_ih.T r|z slices
    wrzhh_d = din("wrzhh", [U, 2 * US], bf16)
    wnih_d = din("wnih", [U, US], bf16)  # W_ih.T n slice
    wnhh_d = din("wnhh", [U, US], bf16)
    brz_hi_d = din("brz_hi", [1, 2 * US], bf16)  # (b_ih+b_hh) r|z hi/lo
    brz_lo_d = din("brz_lo", [1, 2 * US], bf16)
    bin_hi_d = din("bin_hi", [1, US], bf16)  # b_ih n slice hi/lo
    bin_lo_d = din("bin_lo", [1, US], bf16)
    bhn_hi_d = din("bhn_hi", [1, US], bf16)  # b_hh n slice hi/lo
    bhn_lo_d = din("bhn_lo", [1, US], bf16)
    fcw_d = din("fcw", [U, US], f32)  # (fc_w.T * bn_scale) slice
    t2_d = din("t2", [1, US], f32)  # folded fc/BN bias slice
    fc2t_d = din("fc2t", [U, VS], fc2_dt)  # fc2_w.T vocab slice
    if FC2_BF16:
        fc2bh_d = din("fc2bh", [1, VS], bf16)
        fc2bl_d = din("fc2bl", [1, VS], bf16)
    else:
        fc2b_d = din("fc2b", [1, VS], f32)

    logits_d = nc.dram_tensor("logits_c", [B, VS], f32, kind="ExternalOutput")
    hnew_d = nc.dram_tensor("hnew_c", [B, US], f32, kind="ExternalOutput")
    attn_d = nc.dram_tensor("attn_c", [BC, L], f32, kind="ExternalOutput")

    groups = [list(range(N_CORES))]

    with tile.TileContext(nc) as tc:
        from contextlib import ExitStack

        ctxmgr = ExitStack()
        consts = ctxmgr.enter_context(tc.tile_pool(name="consts", bufs=1))
        sb_w = ctxmgr.enter_context(tc.tile_pool(name="weights", bufs=1))
        sb_act = ctxmgr.enter_context(tc.tile_pool(name="acts", bufs=1))
        sb_tz = ctxmgr.enter_context(tc.tile_pool(name="tanhz", bufs=3))
        sb_out = ctxmgr.enter_context(tc.tile_pool(name="outs", bufs=3))
        ps_big = ctxmgr.enter_context(tc.tile_pool(name="ps_big", bufs=2, space="PSUM"))
        ps_sc = ctxmgr.enter_context(tc.tile_pool(name="ps_sc", bufs=2, space="PSUM"))
        ps_sm = ps_big
        dram = ctxmgr.enter_context(tc.tile_pool(name="dram", bufs=1, space="DRAM"))
        fc2_pool = ctxmgr.enter_context(tc.tile_pool(name="fc2s", bufs=24))

        ident32 = consts.tile([128, 128], f32)
        masks.make_identity(nc, ident32[:])
        ident16 = consts.tile([128, 128], bf16)
        masks.make_identity(nc, ident16[:])
        ones16 = consts.tile([1, 128], bf16)
        nc.vector.memset(ones16[:], 1.0)
        ones32 = consts.tile([1, 128], f32)
        nc.vector.memset(ones32[:], 1.0)

        # ---------------- load attention inputs ----------------
        featN = []
        for rt in range(RT):
            t = sb_act.tile([128, EMB], bf16, tag=f"featN{rt}")
            nc.sync.dma_start(t[:], feat16_d[rt * 128 : (rt + 1) * 128, :])
            featN.append(t)
        featT = []
        for ec in range(EC):
            t = sb_act.tile([128, ROWS], bf16, tag=f"featT{ec}")
            nc.sync.dma_start(t[:], featT16_d[ec * 128 : (ec + 1) * 128, :])
            featT.append(t)
        uaT = []
        for ec in range(EC):
            t = sb_w.tile([128, U], bf16, tag=f"uaT{ec}")
            nc.sync.dma_start(t[:], uaT16_d[ec * 128 : (ec + 1) * 128, :])
            uaT.append(t)
        waT = []
        for kc in range(UC):
            t = sb_w.tile([128, U], bf16, tag=f"waT{kc}")
            nc.sync.dma_start(t[:], waT16_d[kc * 128 : (kc + 1) * 128, :])
            waT.append(t)
        hTbc = sb_act.tile([128, UC * BC], bf16)  # [128, 8*16] col-chunked
        nc.sync.dma_start(
            hTbc[:].rearrange("p (kc b) -> p kc b", kc=UC),
            hTbc16_d.rearrange("(kc p) b -> p kc b", p=128),
        )
        sbias = consts.tile([128, UC], f32)
        nc.sync.dma_start(sbias[:], sbias_d[:, :])
        vT16 = consts.tile([128, UC], bf16)
        nc.sync.dma_start(vT16[:], vT16_d[:, :])

        # ---------------- Wh^T [u, b_loc] ----------------
        # Wh = h_c @ Wattn.T as [16, 1024], then PE-transpose to [u, 16].
        wh_sb = sb_act.tile([BC, U], f32)
        for half in range(2):
            ps = ps_big.tile([128, 512], f32, tag="big", name="ps_wh")[:BC, :]
            for kc in range(UC):
                nc.tensor.matmul(
                    ps[:],
                    hTbc[:, kc * BC : (kc + 1) * BC],
                    waT[kc][:, half * 512 : (half + 1) * 512],
                    start=(kc == 0),
                    stop=(kc == UC - 1),
                )
            nc.scalar.activation(
                wh_sb[:, half * 512 : (half + 1) * 512], ps[:], AF.Copy
            )
        whT = sb_act.tile([128, UC * BC], f32)  # [128, 8*16] col-chunked by uc
        for kc in range(UC):
            pst = ps_sm.tile([128, 128], f32, tag="tp", name="ps_whT")[:, :BC]
            nc.tensor.matmul(
                pst[:],
                wh_sb[:, kc * 128 : (kc + 1) * 128],
                ident32[:BC, :BC],
                is_transpose=True,
            )
            nc.vector.tensor_copy(whT[:, kc * BC : (kc + 1) * BC], pst[:])

        # ---------------- Uf + tanh + score ----------------
        # psum_uf[u_chunk, 512 rows] = featT.T @ uaT chunks; add Wh^T broadcast
        # over l; tanh via ACT (bias = Uattn_b+Wattn_b per-partition);
        # score accumulates v^T @ tanhz over u chunks.
        ps_score = []
        for rg in range(2):
            ps_s = ps_sc.tile([1, 512], f32, tag="sc")
            ps_score.append(ps_s)
            for uc in range(UC):
                ps_uf = ps_big.tile([128, 512], f32, tag="big")
                for ec in range(EC):
                    nc.tensor.matmul(
                        ps_uf[:],
                        uaT[ec][:, uc * 128 : (uc + 1) * 128],
                        featT[ec][:, rg * 512 : (rg + 1) * 512],
                        start=(ec == 0),
                        stop=(ec == EC - 1),
                    )
                # add Wh^T: row rg*512+i has b = (rg*512+i)//64
                wh_bcast = (
                    whT[:, uc * BC + rg * 8 : uc * BC + rg * 8 + 8]
                    .unsqueeze(2)
                    .to_broadcast([128, 8, 64])
                )
                nc.vector.tensor_tensor(
                    out=ps_uf[:].rearrange("p (b l) -> p b l", l=64),
                    in0=ps_uf[:].rearrange("p (b l) -> p b l", l=64),
                    in1=wh_bcast,
                    op=OP.add,
                )
                tz = sb_tz.tile([128, 512], bf16, tag="tanhz")
                nc.scalar.activation(
                    tz[:], ps_uf[:], AF.Tanh, bias=sbias[:, uc : uc + 1]
                )
                nc.tensor.matmul(
                    ps_s[:],
                    vT16[:, uc : uc + 1],
                    tz[:],
                    start=(uc == 0),
                    stop=(uc == UC - 1),
                )

        # ---------------- softmax over l (per b) ----------------
        score_sb = sb_act.tile([1, ROWS], f32)
        for rg in range(2):
            nc.scalar.activation(
                score_sb[:, rg * 512 : (rg + 1) * 512], ps_score[rg][:], AF.Copy
            )
        sc3 = score_sb[:].rearrange("p (b l) -> p b l", l=64)
        mx = sb_act.tile([1, BC], f32)
        nc.vector.reduce_max(mx[:], sc3, axis=mybir.AxisListType.X)
        mx_b = mx[:].unsqueeze(2).to_broadcast([1, BC, 64])
        nc.vector.tensor_tensor(out=sc3, in0=sc3, in1=mx_b, op=OP.subtract)
        esb = sb_act.tile([1, ROWS], f32)
        nc.scalar.activation(esb[:], score_sb[:], AF.Exp)
        ssum = sb_act.tile([1, BC], f32)
        nc.vector.reduce_sum(
            ssum[:], esb[:].rearrange("p (b l) -> p b l", l=64), axis=mybir.AxisListType.X
        )
        rsum = sb_act.tile([1, BC], f32)
        nc.vector.reciprocal(rsum[:], ssum[:])
        attn_sb = sb_act.tile([1, ROWS], f32)
        nc.vector.tensor_tensor(
            out=attn_sb[:].rearrange("p (b l) -> p b l", l=64),
            in0=esb[:].rearrange("p (b l) -> p b l", l=64),
            in1=rsum[:].unsqueeze(2).to_broadcast([1, BC, 64]),
            op=OP.mult,
        )
        nc.sync.dma_start(attn_d[:, :], attn_sb[:])
        attn16 = sb_act.tile([1, ROWS], bf16)
        nc.vector.tensor_copy(attn16[:], attn_sb[:])

        # ---------------- ctx = attn-weighted sum of features ----------------
        # Build block "diagonal" lhsT tiles: Ablk[:, rt*16+b] holds attn[b, :]
        # at partitions (b-2rt)*64..+64 for b in {2rt, 2rt+1}, zeros elsewhere.
        ablk = sb_act.tile([128, RT * BC], bf16)
        nc.vector.memset(ablk[:], 0.0)
        for rt in range(RT):
            pst = ps_sm.tile([128, 256], bf16, tag="tp", name="ps_attnT")[:, :1]
            nc.tensor.matmul(
                pst[:],
                attn16[:, rt * 128 : (rt + 1) * 128],
                ident16[:1, :1],
                is_transpose=True,
            )
            b0 = 2 * rt
            nc.vector.tensor_copy(
                ablk[0:64, rt * BC + b0 : rt * BC + b0 + 1], pst[0:64, :]
            )
            nc.vector.tensor_copy(
                ablk[64:128, rt * BC + b0 + 1 : rt * BC + b0 + 2], pst[64:128, :]
            )
        ps_ctx = ps_big.tile([128, 512], f32, tag="big", name="ps_ctx")[:BC, :EMB]
        for rt in range(RT):
            nc.tensor.matmul(
                ps_ctx[:],
                ablk[:, rt * BC : (rt + 1) * BC],
                featN[rt][:],
                start=(rt == 0),
                stop=(rt == RT - 1),
            )
        ctx_sb = sb_act.tile([BC, EMB], f32)
        nc.scalar.activation(ctx_sb[:], ps_ctx[:], AF.Copy)

        # ---------------- AllGather ctx -> full batch ----------------
        ctx_bounce = dram.tile([BC, EMB], f32)
        ctx_ag = dram.tile([B, EMB], f32)
        nc.sync.dma_start(ctx_bounce[:], ctx_sb[:])
        nc.gpsimd.collective_compute(
            "AllGather",
            OP.bypass,
            replica_groups=groups,
            ins=[ctx_bounce.opt()],
            outs=[ctx_ag.opt()],
        )

        # ---------------- ginT (bf16): ctx^T chunks + e^T chunks ----------------
        ginT = sb_act.tile([128, UC * B], bf16)  # col-chunked [kc][128, 128]
        cg_sb = sb_act.tile([128, EMB], f32)
        nc.sync.dma_start(cg_sb[:], ctx_ag[:, :])
        for ec in range(EC):
            pst = ps_sm.tile([128, 128], f32, tag="tp")
            nc.tensor.matmul(
                pst[:],
                cg_sb[:, ec * 128 : (ec + 1) * 128],
                ident32[:, :],
                is_transpose=True,
            )
            nc.vector.tensor_copy(ginT[:, ec * B : (ec + 1) * B], pst[:])
        nc.sync.dma_start(
            ginT[:, EC * B :].rearrange("p (kc b) -> p kc b", kc=EC),
            eT16_d.rearrange("(kc p) b -> p kc b", p=128),
        )
        hT_full = []  # bf16 [128, 128] chunks of h^T for the gh matmuls
        for kc in range(UC):
            t = sb_act.tile([128, B], bf16, tag=f"hTf{kc}")
            nc.sync.dma_start(t[:], hT16_d[kc * 128 : (kc + 1) * 128, :])
            hT_full.append(t)

        # ---------------- GRU gates (TP slice of 128 u per core) ----------------
        wrzih = sb_w.tile([128, UC * 2 * US], bf16)
        nc.sync.dma_start(
            wrzih[:].rearrange("p (kc n) -> p kc n", kc=UC),
            wrzih_d.rearrange("(kc p) n -> p kc n", p=128),
        )
        wrzhh = sb_w.tile([128, UC * 2 * US], bf16)
        nc.sync.dma_start(
            wrzhh[:].rearrange("p (kc n) -> p kc n", kc=UC),
            wrzhh_d.rearrange("(kc p) n -> p kc n", p=128),
        )
        wnih = sb_w.tile([128, UC * US], bf16)
        nc.sync.dma_start(
            wnih[:].rearrange("p (kc n) -> p kc n", kc=UC),
            wnih_d.rearrange("(kc p) n -> p kc n", p=128),
        )
        wnhh = sb_w.tile([128, UC * US], bf16)
        nc.sync.dma_start(
            wnhh[:].rearrange("p (kc n) -> p kc n", kc=UC),
            wnhh_d.rearrange("(kc p) n -> p kc n", p=128),
        )
        brz_hi = sb_w.tile([1, 2 * US], bf16)
        nc.sync.dma_start(brz_hi[:], brz_hi_d[:, :])
        brz_lo = sb_w.tile([1, 2 * US], bf16)
        nc.sync.dma_start(brz_lo[:], brz_lo_d[:, :])
        bin_hi = sb_w.tile([1, US], bf16)
        nc.sync.dma_start(bin_hi[:], bin_hi_d[:, :])
        bin_lo = sb_w.tile([1, US], bf16)
        nc.sync.dma_start(bin_lo[:], bin_lo_d[:, :])
        bhn_hi = sb_w.tile([1, US], bf16)
        nc.sync.dma_start(bhn_hi[:], bhn_hi_d[:, :])
        bhn_lo = sb_w.tile([1, US], bf16)
        nc.sync.dma_start(bhn_lo[:], bhn_lo_d[:, :])

        ps_rz = ps_big.tile([128, 512], f32, tag="big", name="ps_rz")[:, : 2 * US]
        n_mm = 2 * UC + 2
        i_mm = 0
        for kc in range(UC):
            nc.tensor.matmul(
                ps_rz[:],
                ginT[:, kc * B : (kc + 1) * B],
                wrzih[:, kc * 2 * US : (kc + 1) * 2 * US],
                start=(i_mm == 0),
                stop=(i_mm == n_mm - 1),
            )
            i_mm += 1
        for kc in range(UC):
            nc.tensor.matmul(
                ps_rz[:],
                hT_full[kc][:],
                wrzhh[:, kc * 2 * US : (kc + 1) * 2 * US],
                start=(i_mm == 0),
                stop=(i_mm == n_mm - 1),
            )
            i_mm += 1
        nc.tensor.matmul(ps_rz[:], ones16[:], brz_hi[:], start=False, stop=False)
        nc.tensor.matmul(ps_rz[:], ones16[:], brz_lo[:], start=False, stop=True)

        ps_in = ps_sm.tile([128, 128], f32, tag="tp")
        for kc in range(UC):
            nc.tensor.matmul(
                ps_in[:],
                ginT[:, kc * B : (kc + 1) * B],
                wnih[:, kc * US : (kc + 1) * US],
                start=(kc == 0),
                stop=False,
            )
        nc.tensor.matmul(ps_in[:], ones16[:], bin_hi[:], start=False, stop=False)
        nc.tensor.matmul(ps_in[:], ones16[:], bin_lo[:], start=False, stop=True)

        ps_hn = ps_sm.tile([128, 128], f32, tag="tp")
        for kc in range(UC):
            nc.tensor.matmul(
                ps_hn[:],
                hT_full[kc][:],
                wnhh[:, kc * US : (kc + 1) * US],
                start=(kc == 0),
                stop=False,
            )
        nc.tensor.matmul(ps_hn[:], ones16[:], bhn_hi[:], start=False, stop=False)
        nc.tensor.matmul(ps_hn[:], ones16[:], bhn_lo[:], start=False, stop=True)

        r_sb = sb_act.tile([B, US], f32)
        nc.scalar.activation(r_sb[:], ps_rz[:, 0:US], AF.Sigmoid)
        z_sb = sb_act.tile([B, US], f32)
        nc.scalar.activation(z_sb[:], ps_rz[:, US : 2 * US], AF.Sigmoid)
        rhn = sb_act.tile([B, US], f32)
        nc.vector.tensor_tensor(out=rhn[:], in0=ps_hn[:], in1=r_sb[:], op=OP.mult)
        nc.vector.tensor_tensor(out=ps_in[:], in0=ps_in[:], in1=rhn[:], op=OP.add)
        n_sb = sb_act.tile([B, US], f32)
        nc.scalar.activation(n_sb[:], ps_in[:], AF.Tanh)
        hU = sb_act.tile([B, US], f32)
        nc.sync.dma_start(hU[:], hU_d[:, :])
        d_sb = sb_act.tile([B, US], f32)
        nc.vector.tensor_tensor(out=d_sb[:], in0=hU[:], in1=n_sb[:], op=OP.subtract)
        zd_sb = sb_act.tile([B, US], f32)
        nc.vector.tensor_tensor(out=zd_sb[:], in0=z_sb[:], in1=d_sb[:], op=OP.mult)
        hnew_sb = sb_act.tile([B, US], f32)
        nc.vector.tensor_tensor(out=hnew_sb[:], in0=n_sb[:], in1=zd_sb[:], op=OP.add)
        nc.sync.dma_start(hnew_d[:, :], hnew_sb[:])

        # ---------------- AllGather h_new ----------------
        hnew_bounce = dram.tile([B, US], f32)
        hnew_ag = dram.tile([N_CORES * B, US], f32)  # blocks [c][b, u_loc]
        nc.sync.dma_start(hnew_bounce[:], hnew_sb[:])
        nc.gpsimd.collective_compute(
            "AllGather",
            OP.bypass,
            replica_groups=groups,
            ins=[hnew_bounce.opt()],
            outs=[hnew_ag.opt()],
        )

        # transpose blocks to h_new^T [u, b] chunks (f32 for the fc matmul)
        hnT = sb_act.tile([128, UC * B], f32)
        for c2 in range(N_CORES):
            blk = sb_tz.tile([128, US], f32, tag="hn_blk")
            nc.sync.dma_start(blk[:], hnew_ag[c2 * B : (c2 + 1) * B, :])
            pst = ps_sm.tile([128, 128], f32, tag="tp")
            nc.tensor.matmul(pst[:], blk[:], ident32[:, :], is_transpose=True)
            nc.vector.tensor_copy(hnT[:, c2 * B : (c2 + 1) * B], pst[:])

        # ---------------- y = h_new @ fc_w^T (BN folded) ----------------
        fcw = sb_w.tile([128, UC * US], f32)
        nc.sync.dma_start(
            fcw[:].rearrange("p (kc n) -> p kc n", kc=UC),
            fcw_d.rearrange("(kc p) n -> p kc n", p=128),
        )
        t2 = sb_w.tile([1, US], f32)
        nc.sync.dma_start(t2[:], t2_d[:, :])
        ps_y = ps_sm.tile([128, 128], f32, tag="tp")
        for kc in range(UC):
            nc.tensor.matmul(
                ps_y[:],
                hnT[:, kc * B : (kc + 1) * B],
                fcw[:, kc * US : (kc + 1) * US],
                start=(kc == 0),
                stop=False,
            )
        nc.tensor.matmul(ps_y[:], ones32[:], t2[:], start=False, stop=True)
        ybn_sb = sb_act.tile([B, US], f32)
        nc.vector.tensor_copy(ybn_sb[:], ps_y[:])

        # ---------------- AllGather ybn ----------------
        ybn_bounce = dram.tile([B, US], f32)
        ybn_ag = dram.tile([N_CORES * B, US], f32)
        nc.sync.dma_start(ybn_bounce[:], ybn_sb[:])
        nc.gpsimd.collective_compute(
            "AllGather",
            OP.bypass,
            replica_groups=groups,
            ins=[ybn_bounce.opt()],
            outs=[ybn_ag.opt()],
        )
        fc2_in_dt = bf16 if FC2_BF16 else f32
        ybnT = sb_act.tile([128, UC * B], fc2_in_dt)
        for c2 in range(N_CORES):
            blk = sb_tz.tile([128, US], f32, tag="ybn_blk")
            nc.sync.dma_start(blk[:], ybn_ag[c2 * B : (c2 + 1) * B, :])
            pst = ps_sm.tile([128, 128], f32, tag="tp")
            nc.tensor.matmul(pst[:], blk[:], ident32[:, :], is_transpose=True)
            nc.vector.tensor_copy(ybnT[:, c2 * B : (c2 + 1) * B], pst[:])

        # ---------------- fc2: logits = ybn @ fc2_w^T + fc2_b ----------------
        if FC2_BF16:
            fc2bh = sb_w.tile([1, VS], bf16)
            nc.sync.dma_start(fc2bh[:], fc2bh_d[:, :])
            fc2bl = sb_w.tile([1, VS], bf16)
            nc.sync.dma_start(fc2bl[:], fc2bl_d[:, :])
        else:
            fc2b = sb_w.tile([1, VS], f32)
            nc.sync.dma_start(fc2b[:], fc2b_d[:, :])
        for vc in range(NVC):
            nv = min(512, VS - vc * 512)
            ps_l = ps_big.tile([128, 512], f32, tag="big")
            for kc in range(UC):
                wt = fc2_pool.tile([128, 512], fc2_in_dt, tag="fc2w")
                nc.sync.dma_start(
                    wt[:, :nv],
                    fc2t_d[kc * 128 : (kc + 1) * 128, vc * 512 : vc * 512 + nv],
                )
                nc.tensor.matmul(
                    ps_l[:, :nv],
                    ybnT[:, kc * B : (kc + 1) * B],
                    wt[:, :nv],
                    start=(kc == 0),
                    stop=False,
                )
            if FC2_BF16:
                nc.tensor.matmul(
                    ps_l[:, :nv],
                    ones16[:],
                    fc2bh[:, vc * 512 : vc * 512 + nv],
                    start=False,
                    stop=False,
                )
                nc.tensor.matmul(
                    ps_l[:, :nv],
                    ones16[:],
                    fc2bl[:, vc * 512 : vc * 512 + nv],
                    start=False,
                    stop=True,
                )
            else:
                nc.tensor.matmul(
                    ps_l[:, :nv],
                    ones32[:],
                    fc2b[:, vc * 512 : vc * 512 + nv],
                    start=False,
                    stop=True,
                )
            lg = sb_out.tile([B, 512], f32, tag="lg")
            nc.vector.tensor_copy(lg[:, :nv], ps_l[:, :nv])
            nc.sync.dma_start(logits_d[:, vc * 512 : vc * 512 + nv], lg[:, :nv])

        ctxmgr.close()

    nc.compile()
    return nc


def _host_prep(inputs):
    import ml_dtypes

    bf16 = ml_dtypes.bfloat16
    f32 = np.float32

    x = np.asarray(inputs["x"])
    features = np.asarray(inputs["features"], dtype=f32)
    hidden = np.asarray(inputs["hidden"], dtype=f32)
    emb_table = np.asarray(inputs["emb_table"], dtype=f32)
    Uattn_w = np.asarray(inputs["Uattn_w"], dtype=f32)
    Uattn_b = np.asarray(inputs["Uattn_b"], dtype=f32)
    Wattn_w = np.asarray(inputs["Wattn_w"], dtype=f32)
    Wattn_b = np.asarray(inputs["Wattn_b"], dtype=f32)
    Vattn_w = np.asarray(inputs["Vattn_w"], dtype=f32)
    W_ih = np.asarray(inputs["W_ih"], dtype=f32)
    W_hh = np.asarray(inputs["W_hh"], dtype=f32)
    b_ih = np.asarray(inputs["b_ih"], dtype=f32)
    b_hh = np.asarray(inputs["b_hh"], dtype=f32)
    fc_w = np.asarray(inputs["fc_w"], dtype=f32)
    fc_b = np.asarray(inputs["fc_b"], dtype=f32)
    bn_gamma = np.asarray(inputs["bn_gamma"], dtype=f32)
    bn_beta = np.asarray(inputs["bn_beta"], dtype=f32)
    bn_mean = np.asarray(inputs["bn_mean"], dtype=f32)
    bn_var = np.asarray(inputs["bn_var"], dtype=f32)
    fc2_w = np.asarray(inputs["fc2_w"], dtype=f32)
    fc2_b = np.asarray(inputs["fc2_b"], dtype=f32)

    h = hidden[0]  # [B, U]
    hT16 = np.ascontiguousarray(h.T).astype(bf16)  # [U, B]
    uaT16 = np.ascontiguousarray(Uattn_w.T).astype(bf16)  # [EMB, U]
    waT16 = np.ascontiguousarray(Wattn_w.T).astype(bf16)  # [U, U]
    sbias = np.ascontiguousarray((Uattn_b + Wattn_b).reshape(UC, 128).T)  # [128, UC]
    vT16 = np.ascontiguousarray(Vattn_w[0].reshape(UC, 128).T).astype(bf16)
    e_full = emb_table[x[:, 0].astype(np.int64)]  # [B, EMB] row gather
    eT16 = np.ascontiguousarray(e_full.T).astype(bf16)  # [EMB, B]

    W_ihT = np.ascontiguousarray(W_ih.T)  # [U, 3U]
    W_hhT = np.ascontiguousarray(W_hh.T)
    bsum = (b_ih.astype(np.float64) + b_hh.astype(np.float64))  # [3U]

    bn_s = bn_gamma / np.sqrt(bn_var + BN_EPS)
    bn_t = bn_beta - bn_mean * bn_s
    fc_wTs = np.ascontiguousarray(fc_w.T * bn_s[None, :])  # [U, U]
    t2_full = (fc_b * bn_s + bn_t).astype(f32)  # [U]

    fc2T = np.zeros((U, N_CORES * VS), dtype=f32)
    fc2T[:, :V] = fc2_w.T
    fc2b_pad = np.zeros(N_CORES * VS, dtype=np.float64)
    fc2b_pad[:V] = fc2_b.astype(np.float64)

    def hilo(v):  # bf16 hi/lo split of a float64 vector -> [1, n] each
        hi = v.astype(bf16)
        lo = (v - hi.astype(np.float64)).astype(bf16)
        return hi.reshape(1, -1), lo.reshape(1, -1)

    in_maps = []
    for c in range(N_CORES):
        bc = slice(c * BC, (c + 1) * BC)
        Sc = slice(c * US, (c + 1) * US)
        feat_c = np.ascontiguousarray(features[bc].reshape(ROWS, EMB))
        m = {
            "feat16": feat_c.astype(bf16),
            "featT16": np.ascontiguousarray(feat_c.T).astype(bf16),
            "uaT16": uaT16,
            "waT16": waT16,
            "hT16": hT16,
            "hTbc16": np.ascontiguousarray(hT16[:, bc]),
            "hU": np.ascontiguousarray(h[:, Sc]),
            "sbias": sbias,
            "vT16": vT16,
            "eT16": eT16,
            "wrzih": np.ascontiguousarray(
                np.hstack([W_ihT[:, Sc], W_ihT[:, U + c * US : U + (c + 1) * US]])
            ).astype(bf16),
            "wrzhh": np.ascontiguousarray(
                np.hstack([W_hhT[:, Sc], W_hhT[:, U + c * US : U + (c + 1) * US]])
            ).astype(bf16),
            "wnih": np.ascontiguousarray(
                W_ihT[:, 2 * U + c * US : 2 * U + (c + 1) * US]
            ).astype(bf16),
            "wnhh": np.ascontiguousarray(
                W_hhT[:, 2 * U + c * US : 2 * U + (c + 1) * US]
            ).astype(bf16),
            "fcw": np.ascontiguousarray(fc_wTs[:, Sc]),
            "t2": t2_full[Sc].reshape(1, US),
            "fc2t": np.ascontiguousarray(fc2T[:, c * VS : (c + 1) * VS]).astype(
                bf16 if FC2_BF16 else f32
            ),
        }
        brz = np.concatenate([bsum[Sc], bsum[U + c * US : U + (c + 1) * US]])
        m["brz_hi"], m["brz_lo"] = hilo(brz)
        m["bin_hi"], m["bin_lo"] = hilo(
            b_ih[2 * U + c * US : 2 * U + (c + 1) * US].astype(np.float64)
        )
        m["bhn_hi"], m["bhn_lo"] = hilo(
            b_hh[2 * U + c * US : 2 * U + (c + 1) * US].astype(np.float64)
        )
        fb = fc2b_pad[c * VS : (c + 1) * VS]
        if FC2_BF16:
            m["fc2bh"], m["fc2bl"] = hilo(fb)
        else:
            m["fc2b"] = fb.astype(f32).reshape(1, VS)
        in_maps.append(m)
    return in_maps


def kernel(**inputs):
    from concourse.bass_utils import run_bass_kernel_spmd

    if "nc" not in _CACHE:
        _CACHE["nc"] = _build_bass()
    nc = _CACHE["nc"]
    in_maps = _host_prep(inputs)
    res = run_bass_kernel_spmd(nc, in_maps, list(range(N_CORES)))
    rs = res.results
    logits = np.concatenate([rs[c]["logits_c"] for c in range(N_CORES)], axis=1)[
        :, :V
    ]
    h_new = np.concatenate([rs[c]["hnew_c"] for c in range(N_CORES)], axis=1)[
        None, :, :
    ]
    attn = np.concatenate([rs[c]["attn_c"] for c in range(N_CORES)], axis=0)[
        :, :, None
    ]
    return logits.astype(np.float32), h_new.astype(np.float32), attn.astype(
        np.float32
    )


# revision 8
# speedup vs baseline: 1.3138x; 1.3138x over previous
"""TRN2 Bass kernel for nn_Decode (Bahdanau-attention GRU decode step + vocab head).

Sharding across 8 NeuronCores (SPMD, one program, per-core data):
  - Attention: data-parallel over batch (16 rows/core); small attention weights
    replicated (shipped bf16).
  - GRU + fc/BN: tensor-parallel over the hidden dim (128 of 1024 per core),
    stitched with AllGather collectives.
  - fc2 (vocab head) + embedding gather: tensor-parallel over vocab
    (6283 of 50257 rows per core).
  - Host does layout prep only: weight transposes/casts, embedding row gather,
    BN constant folding, output concatenation.

Numerics: bf16 operands for attention/GRU matmuls (≈1e-4 relative error on
outputs), fp32 for the fc/BN/fc2 chain unless FC2_BF16 is set.
"""

import numpy as np

N_CORES = 8
B, L, EMB, U, V = 128, 64, 512, 1024, 50257
BC = B // N_CORES  # 16 batch rows per core
US = U // N_CORES  # 128 hidden slice per core (GRU/fc TP)
VS = -(-V // N_CORES)  # 6283 vocab rows per core
BN_EPS = 1e-3

FC2_BF16 = True  # fc2 weights/matmul in bf16 (halves the dominant DMA)

ROWS = BC * L  # 1024 attention rows per core
RT = ROWS // 128  # 8 row chunks
EC = EMB // 128  # 4 embedding chunks
UC = U // 128  # 8 hidden chunks
NVC = -(-VS // 512)  # 13 vocab column chunks (last is 139 wide)

_CACHE = {}


def _build_bass():
    import concourse.tile as tile
    from concourse import bacc, masks, mybir

    f32 = mybir.dt.float32
    bf16 = mybir.dt.bfloat16
    AF = mybir.ActivationFunctionType
    OP = mybir.AluOpType
    fc2_dt = bf16 if FC2_BF16 else f32

    nc = bacc.Bacc(None, target_bir_lowering=False, num_devices=N_CORES)

    din = lambda n, s, d: nc.dram_tensor(n, s, d, kind="ExternalInput")
    feat16_d = din("feat16", [ROWS, EMB], bf16)  # this core's features, natural
    featT16_d = din("featT16", [EMB, ROWS], bf16)  # transposed
    uaT16_d = din("uaT16", [EMB, U], bf16)  # Uattn_w.T (replicated)
    waT16_d = din("waT16", [U, U], bf16)  # Wattn_w.T (replicated)
    hT16_d = din("hT16", [U, B], bf16)  # full h.T (replicated)
    hTbc16_d = din("hTbc16", [U, BC], bf16)  # this core's batch cols of h.T
    hU_d = din("hU", [B, US], f32)  # this core's hidden-slice of h, natural
    sbias_d = din("sbias", [128, UC], f32)  # (Uattn_b+Wattn_b) col-chunked
    vT16_d = din("vT16", [128, UC], bf16)  # Vattn_w col-chunked
    eT16_d = din("eT16", [EMB, B], bf16)  # gathered embeddings.T (replicated)
    wrzih_d = din("wrzih", [U, 2 * US], bf16)  # W_ih.T r|z slices
    wrzhh_d = din("wrzhh", [U, 2 * US], bf16)
    wnih_d = din("wnih", [U, US], bf16)  # W_ih.T n slice
    wnhh_d = din("wnhh", [U, US], bf16)
    brz_hi_d = din("brz_hi", [1, 2 * US], bf16)  # (b_ih+b_hh) r|z hi/lo
    brz_lo_d = din("brz_lo", [1, 2 * US], bf16)
    bin_hi_d = din("bin_hi", [1, US], bf16)  # b_ih n slice hi/lo
    bin_lo_d = din("bin_lo", [1, US], bf16)
    bhn_hi_d = din("bhn_hi", [1, US], bf16)  # b_hh n slice hi/lo
    bhn_lo_d = din("bhn_lo", [1, US], bf16)
    fcw_d = din("fcw", [U, U], bf16)  # (fc_w.T * bn_scale), full, replicated
    t2h_d = din("t2h", [1, U], bf16)  # folded fc/BN bias hi/lo
    t2l_d = din("t2l", [1, U], bf16)
    fc2t_d = din("fc2t", [U, VS], fc2_dt)  # fc2_w.T vocab slice
    if FC2_BF16:
        fc2bh_d = din("fc2bh", [1, VS], bf16)
        fc2bl_d = din("fc2bl", [1, VS], bf16)
    else:
        fc2b_d = din("fc2b", [1, VS], f32)

    logits_d = nc.dram_tensor("logits_c", [B, VS], f32, kind="ExternalOutput")
    hnew_d = nc.dram_tensor("hnew_c", [B, US], f32, kind="ExternalOutput")
    attn_d = nc.dram_tensor("attn_c", [BC, L], f32, kind="ExternalOutput")

    groups = [list(range(N_CORES))]

    with tile.TileContext(nc) as tc:
        from contextlib import ExitStack

        ctxmgr = ExitStack()
        consts = ctxmgr.enter_context(tc.tile_pool(name="consts", bufs=1))
        sb_w = ctxmgr.enter_context(tc.tile_pool(name="weights", bufs=1))
        sb_act = ctxmgr.enter_context(tc.tile_pool(name="acts", bufs=1))
        sb_tz = ctxmgr.enter_context(tc.tile_pool(name="tanhz", bufs=3))
        sb_out = ctxmgr.enter_context(tc.tile_pool(name="outs", bufs=3))
        ps_big = ctxmgr.enter_context(tc.tile_pool(name="ps_big", bufs=2, space="PSUM"))
        ps_sc = ctxmgr.enter_context(tc.tile_pool(name="ps_sc", bufs=2, space="PSUM"))
        ps_sm = ps_big
        dram = ctxmgr.enter_context(tc.tile_pool(name="dram", bufs=1, space="DRAM"))
        fc2_pool = ctxmgr.enter_context(tc.tile_pool(name="fc2s", bufs=24))

        ident32 = consts.tile([128, 128], f32)
        masks.make_identity(nc, ident32[:])
        ident16 = consts.tile([128, 128], bf16)
        masks.make_identity(nc, ident16[:])
        ones16 = consts.tile([1, 128], bf16)
        nc.vector.memset(ones16[:], 1.0)
        ones32 = consts.tile([1, 128], f32)
        nc.vector.memset(ones32[:], 1.0)

        # ---------------- load attention inputs ----------------
        featN = []
        for rt in range(RT):
            t = sb_act.tile([128, EMB], bf16, tag=f"featN{rt}")
            nc.sync.dma_start(t[:], feat16_d[rt * 128 : (rt + 1) * 128, :])
            featN.append(t)
        featT = []
        for ec in range(EC):
            t = sb_act.tile([128, ROWS], bf16, tag=f"featT{ec}")
            nc.sync.dma_start(t[:], featT16_d[ec * 128 : (ec + 1) * 128, :])
            featT.append(t)
        uaT = []
        for ec in range(EC):
            t = sb_w.tile([128, U], bf16, tag=f"uaT{ec}")
            nc.sync.dma_start(t[:], uaT16_d[ec * 128 : (ec + 1) * 128, :])
            uaT.append(t)
        waT = []
        for kc in range(UC):
            t = sb_w.tile([128, U], bf16, tag=f"waT{kc}")
            nc.sync.dma_start(t[:], waT16_d[kc * 128 : (kc + 1) * 128, :])
            waT.append(t)
        hTbc = sb_act.tile([128, UC * BC], bf16)  # [128, 8*16] col-chunked
        nc.sync.dma_start(
            hTbc[:].rearrange("p (kc b) -> p kc b", kc=UC),
            hTbc16_d.rearrange("(kc p) b -> p kc b", p=128),
        )
        sbias = consts.tile([128, UC], f32)
        nc.sync.dma_start(sbias[:], sbias_d[:, :])
        vT16 = consts.tile([128, UC], bf16)
        nc.sync.dma_start(vT16[:], vT16_d[:, :])

        # ---------------- Wh^T [u, b_loc] ----------------
        # Wh = h_c @ Wattn.T as [16, 1024], then PE-transpose to [u, 16].
        wh_sb = sb_act.tile([BC, U], f32)
        for half in range(2):
            ps = ps_big.tile([128, 512], f32, tag="big", name="ps_wh")[:BC, :]
            for kc in range(UC):
                nc.tensor.matmul(
                    ps[:],
                    hTbc[:, kc * BC : (kc + 1) * BC],
                    waT[kc][:, half * 512 : (half + 1) * 512],
                    start=(kc == 0),
                    stop=(kc == UC - 1),
                )
            nc.scalar.activation(
                wh_sb[:, half * 512 : (half + 1) * 512], ps[:], AF.Copy
            )
        whT = sb_act.tile([128, UC * BC], f32)  # [128, 8*16] col-chunked by uc
        for kc in range(UC):
            pst = ps_sm.tile([128, 128], f32, tag="tp", name="ps_whT")[:, :BC]
            nc.tensor.matmul(
                pst[:],
                wh_sb[:, kc * 128 : (kc + 1) * 128],
                ident32[:BC, :BC],
                is_transpose=True,
            )
            nc.vector.tensor_copy(whT[:, kc * BC : (kc + 1) * BC], pst[:])

        # ---------------- Uf + tanh + score ----------------
        # psum_uf[u_chunk, 512 rows] = featT.T @ uaT chunks; add Wh^T broadcast
        # over l; tanh via ACT (bias = Uattn_b+Wattn_b per-partition);
        # score accumulates v^T @ tanhz over u chunks.
        ps_score = []
        for rg in range(2):
            ps_s = ps_sc.tile([1, 512], f32, tag="sc")
            ps_score.append(ps_s)
            for uc in range(UC):
                ps_uf = ps_big.tile([128, 512], f32, tag="big")
                for ec in range(EC):
                    nc.tensor.matmul(
                        ps_uf[:],
                        uaT[ec][:, uc * 128 : (uc + 1) * 128],
                        featT[ec][:, rg * 512 : (rg + 1) * 512],
                        start=(ec == 0),
                        stop=(ec == EC - 1),
                    )
                # add Wh^T: row rg*512+i has b = (rg*512+i)//64
                wh_bcast = (
                    whT[:, uc * BC + rg * 8 : uc * BC + rg * 8 + 8]
                    .unsqueeze(2)
                    .to_broadcast([128, 8, 64])
                )
                nc.vector.tensor_tensor(
                    out=ps_uf[:].rearrange("p (b l) -> p b l", l=64),
                    in0=ps_uf[:].rearrange("p (b l) -> p b l", l=64),
                    in1=wh_bcast,
                    op=OP.add,
                )
                tz = sb_tz.tile([128, 512], bf16, tag="tanhz")
                nc.scalar.activation(
                    tz[:], ps_uf[:], AF.Tanh, bias=sbias[:, uc : uc + 1]
                )
                nc.tensor.matmul(
                    ps_s[:],
                    vT16[:, uc : uc + 1],
                    tz[:],
                    start=(uc == 0),
                    stop=(uc == UC - 1),
                )

        # ---------------- softmax over l (per b) ----------------
        score_sb = sb_act.tile([1, ROWS], f32)
        for rg in range(2):
            nc.scalar.activation(
                score_sb[:, rg * 512 : (rg + 1) * 512], ps_score[rg][:], AF.Copy
            )
        sc3 = score_sb[:].rearrange("p (b l) -> p b l", l=64)
        mx = sb_act.tile([1, BC], f32)
        nc.vector.reduce_max(mx[:], sc3, axis=mybir.AxisListType.X)
        mx_b = mx[:].unsqueeze(2).to_broadcast([1, BC, 64])
        nc.vector.tensor_tensor(out=sc3, in0=sc3, in1=mx_b, op=OP.subtract)
        esb = sb_act.tile([1, ROWS], f32)
        nc.scalar.activation(esb[:], score_sb[:], AF.Exp)
        ssum = sb_act.tile([1, BC], f32)
        nc.vector.reduce_sum(
            ssum[:], esb[:].rearrange("p (b l) -> p b l", l=64), axis=mybir.AxisListType.X
        )
        rsum = sb_act.tile([1, BC], f32)
        nc.vector.reciprocal(rsum[:], ssum[:])
        attn_sb = sb_act.tile([1, ROWS], f32)
        nc.vector.tensor_tensor(
            out=attn_sb[:].rearrange("p (b l) -> p b l", l=64),
            in0=esb[:].rearrange("p (b l) -> p b l", l=64),
            in1=rsum[:].unsqueeze(2).to_broadcast([1, BC, 64]),
            op=OP.mult,
        )
        nc.sync.dma_start(attn_d[:, :], attn_sb[:])
        attn16 = sb_act.tile([1, ROWS], bf16)
        nc.vector.tensor_copy(attn16[:], attn_sb[:])

        # ---------------- ctx = attn-weighted sum of features ----------------
        # Build block "diagonal" lhsT tiles: Ablk[:, rt*16+b] holds attn[b, :]
        # at partitions (b-2rt)*64..+64 for b in {2rt, 2rt+1}, zeros elsewhere.
        ablk = sb_act.tile([128, RT * BC], bf16)
        nc.vector.memset(ablk[:], 0.0)
        for rt in range(RT):
            pst = ps_sm.tile([128, 256], bf16, tag="tp", name="ps_attnT")[:, :1]
            nc.tensor.matmul(
                pst[:],
                attn16[:, rt * 128 : (rt + 1) * 128],
                ident16[:1, :1],
                is_transpose=True,
            )
            b0 = 2 * rt
            nc.vector.tensor_copy(
                ablk[0:64, rt * BC + b0 : rt * BC + b0 + 1], pst[0:64, :]
            )
            nc.vector.tensor_copy(
                ablk[64:128, rt * BC + b0 + 1 : rt * BC + b0 + 2], pst[64:128, :]
            )
        ps_ctx = ps_big.tile([128, 512], f32, tag="big", name="ps_ctx")[:BC, :EMB]
        for rt in range(RT):
            nc.tensor.matmul(
                ps_ctx[:],
                ablk[:, rt * BC : (rt + 1) * BC],
                featN[rt][:],
                start=(rt == 0),
                stop=(rt == RT - 1),
            )
        ctx_sb = sb_act.tile([BC, EMB], f32)
        nc.scalar.activation(ctx_sb[:], ps_ctx[:], AF.Copy)

        # ---------------- AllGather ctx -> full batch ----------------
        ctx_bounce = dram.tile([BC, EMB], f32)
        ctx_ag = dram.tile([B, EMB], f32)
        nc.sync.dma_start(ctx_bounce[:], ctx_sb[:])
        nc.gpsimd.collective_compute(
            "AllGather",
            OP.bypass,
            replica_groups=groups,
            ins=[ctx_bounce.opt()],
            outs=[ctx_ag.opt()],
        )

        # ---------------- ginT (bf16): ctx^T chunks + e^T chunks ----------------
        ginT = sb_act.tile([128, UC * B], bf16)  # col-chunked [kc][128, 128]
        cg_sb = sb_act.tile([128, EMB], f32)
        nc.sync.dma_start(cg_sb[:], ctx_ag[:, :])
        for ec in range(EC):
            pst = ps_sm.tile([128, 128], f32, tag="tp")
            nc.tensor.matmul(
                pst[:],
                cg_sb[:, ec * 128 : (ec + 1) * 128],
                ident32[:, :],
                is_transpose=True,
            )
            nc.vector.tensor_copy(ginT[:, ec * B : (ec + 1) * B], pst[:])
        nc.sync.dma_start(
            ginT[:, EC * B :].rearrange("p (kc b) -> p kc b", kc=EC),
            eT16_d.rearrange("(kc p) b -> p kc b", p=128),
        )
        hT_full = []  # bf16 [128, 128] chunks of h^T for the gh matmuls
        for kc in range(UC):
            t = sb_act.tile([128, B], bf16, tag=f"hTf{kc}")
            nc.sync.dma_start(t[:], hT16_d[kc * 128 : (kc + 1) * 128, :])
            hT_full.append(t)

        # ---------------- GRU gates (TP slice of 128 u per core) ----------------
        wrzih = sb_w.tile([128, UC * 2 * US], bf16)
        nc.sync.dma_start(
            wrzih[:].rearrange("p (kc n) -> p kc n", kc=UC),
            wrzih_d.rearrange("(kc p) n -> p kc n", p=128),
        )
        wrzhh = sb_w.tile([128, UC * 2 * US], bf16)
        nc.sync.dma_start(
            wrzhh[:].rearrange("p (kc n) -> p kc n", kc=UC),
            wrzhh_d.rearrange("(kc p) n -> p kc n", p=128),
        )
        wnih = sb_w.tile([128, UC * US], bf16)
        nc.sync.dma_start(
            wnih[:].rearrange("p (kc n) -> p kc n", kc=UC),
            wnih_d.rearrange("(kc p) n -> p kc n", p=128),
        )
        wnhh = sb_w.tile([128, UC * US], bf16)
        nc.sync.dma_start(
            wnhh[:].rearrange("p (kc n) -> p kc n", kc=UC),
            wnhh_d.rearrange("(kc p) n -> p kc n", p=128),
        )
        brz_hi = sb_w.tile([1, 2 * US], bf16)
        nc.sync.dma_start(brz_hi[:], brz_hi_d[:, :])
        brz_lo = sb_w.tile([1, 2 * US], bf16)
        nc.sync.dma_start(brz_lo[:], brz_lo_d[:, :])
        bin_hi = sb_w.tile([1, US], bf16)
        nc.sync.dma_start(bin_hi[:], bin_hi_d[:, :])
        bin_lo = sb_w.tile([1, US], bf16)
        nc.sync.dma_start(bin_lo[:], bin_lo_d[:, :])
        bhn_hi = sb_w.tile([1, US], bf16)
        nc.sync.dma_start(bhn_hi[:], bhn_hi_d[:, :])
        bhn_lo = sb_w.tile([1, US], bf16)
        nc.sync.dma_start(bhn_lo[:], bhn_lo_d[:, :])

        ps_rz = ps_big.tile([128, 512], f32, tag="big", name="ps_rz")[:, : 2 * US]
        n_mm = 2 * UC + 2
        i_mm = 0
        for kc in range(UC):
            nc.tensor.matmul(
                ps_rz[:],
                hT_full[kc][:],
                wrzhh[:, kc * 2 * US : (kc + 1) * 2 * US],
                start=(i_mm == 0),
                stop=(i_mm == n_mm - 1),
            )
            i_mm += 1
        for kc in range(UC):
            nc.tensor.matmul(
                ps_rz[:],
                ginT[:, kc * B : (kc + 1) * B],
                wrzih[:, kc * 2 * US : (kc + 1) * 2 * US],
                start=(i_mm == 0),
                stop=(i_mm == n_mm - 1),
            )
            i_mm += 1
        nc.tensor.matmul(ps_rz[:], ones16[:], brz_hi[:], start=False, stop=False)
        nc.tensor.matmul(ps_rz[:], ones16[:], brz_lo[:], start=False, stop=True)

        ps_hn = ps_sm.tile([128, 128], f32, tag="tp")
        for kc in range(UC):
            nc.tensor.matmul(
                ps_hn[:],
                hT_full[kc][:],
                wnhh[:, kc * US : (kc + 1) * US],
                start=(kc == 0),
                stop=False,
            )
        nc.tensor.matmul(ps_hn[:], ones16[:], bhn_hi[:], start=False, stop=False)
        nc.tensor.matmul(ps_hn[:], ones16[:], bhn_lo[:], start=False, stop=True)

        ps_in = ps_sm.tile([128, 128], f32, tag="tp")
        for kc in range(UC):
            nc.tensor.matmul(
                ps_in[:],
                ginT[:, kc * B : (kc + 1) * B],
                wnih[:, kc * US : (kc + 1) * US],
                start=(kc == 0),
                stop=False,
            )
        nc.tensor.matmul(ps_in[:], ones16[:], bin_hi[:], start=False, stop=False)
        nc.tensor.matmul(ps_in[:], ones16[:], bin_lo[:], start=False, stop=True)

        r_sb = sb_act.tile([B, US], f32)
        nc.scalar.activation(r_sb[:], ps_rz[:, 0:US], AF.Sigmoid)
        z_sb = sb_act.tile([B, US], f32)
        nc.scalar.activation(z_sb[:], ps_rz[:, US : 2 * US], AF.Sigmoid)
        rhn = sb_act.tile([B, US], f32)
        nc.vector.tensor_tensor(out=rhn[:], in0=ps_hn[:], in1=r_sb[:], op=OP.mult)
        nc.vector.tensor_tensor(out=ps_in[:], in0=ps_in[:], in1=rhn[:], op=OP.add)
        n_sb = sb_act.tile([B, US], f32)
        nc.scalar.activation(n_sb[:], ps_in[:], AF.Tanh)
        hU = sb_act.tile([B, US], f32)
        nc.sync.dma_start(hU[:], hU_d[:, :])
        d_sb = sb_act.tile([B, US], f32)
        nc.vector.tensor_tensor(out=d_sb[:], in0=hU[:], in1=n_sb[:], op=OP.subtract)
        zd_sb = sb_act.tile([B, US], f32)
        nc.vector.tensor_tensor(out=zd_sb[:], in0=z_sb[:], in1=d_sb[:], op=OP.mult)
        hnew_sb = sb_act.tile([B, US], f32)
        nc.vector.tensor_tensor(out=hnew_sb[:], in0=n_sb[:], in1=zd_sb[:], op=OP.add)
        nc.sync.dma_start(hnew_d[:, :], hnew_sb[:])

        # ---------------- AllGather h_new ----------------
        hnew_bounce = dram.tile([B, US], f32)
        hnew_ag = dram.tile([N_CORES * B, US], f32)  # blocks [c][b, u_loc]
        nc.sync.dma_start(hnew_bounce[:], hnew_sb[:])
        nc.gpsimd.collective_compute(
            "AllGather",
            OP.bypass,
            replica_groups=groups,
            ins=[hnew_bounce.opt()],
            outs=[hnew_ag.opt()],
        )

        # transpose blocks to h_new^T [u, b] chunks (f32 for the fc matmul)
        hnT = sb_act.tile([128, UC * B], bf16)
        for c2 in range(N_CORES):
            blk = sb_tz.tile([128, US], f32, tag="hn_blk")
            nc.sync.dma_start(blk[:], hnew_ag[c2 * B : (c2 + 1) * B, :])
            pst = ps_sm.tile([128, 128], f32, tag="tp")
            nc.tensor.matmul(pst[:], blk[:], ident32[:, :], is_transpose=True)
            nc.vector.tensor_copy(hnT[:, c2 * B : (c2 + 1) * B], pst[:])

        # ---------------- y = h_new @ fc_w^T (BN folded), replicated ----------------
        fcw = sb_w.tile([128, UC * U], bf16)
        nc.sync.dma_start(
            fcw[:].rearrange("p (kc n) -> p kc n", kc=UC),
            fcw_d.rearrange("(kc p) n -> p kc n", p=128),
        )
        t2h = sb_w.tile([1, U], bf16)
        nc.sync.dma_start(t2h[:], t2h_d[:, :])
        t2l = sb_w.tile([1, U], bf16)
        nc.sync.dma_start(t2l[:], t2l_d[:, :])
        ybn_sb = sb_act.tile([B, U], f32)
        for half in range(2):
            ps_y = ps_big.tile([128, 512], f32, tag="big", name="ps_y")
            for kc in range(UC):
                nc.tensor.matmul(
                    ps_y[:],
                    hnT[:, kc * B : (kc + 1) * B],
                    fcw[:, kc * U + half * 512 : kc * U + (half + 1) * 512],
                    start=(kc == 0),
                    stop=False,
                )
            nc.tensor.matmul(
                ps_y[:], ones16[:], t2h[:, half * 512 : (half + 1) * 512],
                start=False, stop=False,
            )
            nc.tensor.matmul(
                ps_y[:], ones16[:], t2l[:, half * 512 : (half + 1) * 512],
                start=False, stop=True,
            )
            nc.vector.tensor_copy(ybn_sb[:, half * 512 : (half + 1) * 512], ps_y[:])

        fc2_in_dt = bf16 if FC2_BF16 else f32
        ybnT = sb_act.tile([128, UC * B], fc2_in_dt)
        for c2 in range(N_CORES):
            pst = ps_sm.tile([128, 128], f32, tag="tp")
            nc.tensor.matmul(
                pst[:], ybn_sb[:, c2 * US : (c2 + 1) * US], ident32[:, :],
                is_transpose=True,
            )
            nc.vector.tensor_copy(ybnT[:, c2 * B : (c2 + 1) * B], pst[:])

        # ---------------- fc2: logits = ybn @ fc2_w^T + fc2_b ----------------
        if FC2_BF16:
            fc2bh = sb_w.tile([1, VS], bf16)
            nc.sync.dma_start(fc2bh[:], fc2bh_d[:, :])
            fc2bl = sb_w.tile([1, VS], bf16)
            nc.sync.dma_start(fc2bl[:], fc2bl_d[:, :])
        else:
            fc2b = sb_w.tile([1, VS], f32)
            nc.sync.dma_start(fc2b[:], fc2b_d[:, :])
        for vc in range(NVC):
            nv = min(512, VS - vc * 512)
            ps_l = ps_big.tile([128, 512], f32, tag="big")
            for kc in range(UC):
                wt = fc2_pool.tile([128, 512], fc2_in_dt, tag="fc2w")
                nc.sync.dma_start(
                    wt[:, :nv],
                    fc2t_d[kc * 128 : (kc + 1) * 128, vc * 512 : vc * 512 + nv],
                )
                nc.tensor.matmul(
                    ps_l[:, :nv],
                    ybnT[:, kc * B : (kc + 1) * B],
                    wt[:, :nv],
                    start=(kc == 0),
                    stop=False,
                )
            if FC2_BF16:
                nc.tensor.matmul(
                    ps_l[:, :nv],
                    ones16[:],
                    fc2bh[:, vc * 512 : vc * 512 + nv],
                    start=False,
                    stop=False,
                )
                nc.tensor.matmul(
                    ps_l[:, :nv],
                    ones16[:],
                    fc2bl[:, vc * 512 : vc * 512 + nv],
                    start=False,
                    stop=True,
                )
            else:
                nc.tensor.matmul(
                    ps_l[:, :nv],
                    ones32[:],
                    fc2b[:, vc * 512 : vc * 512 + nv],
                    start=False,
                    stop=True,
                )
            lg = sb_out.tile([B, 512], f32, tag="lg")
            nc.vector.tensor_copy(lg[:, :nv], ps_l[:, :nv])
            nc.sync.dma_start(logits_d[:, vc * 512 : vc * 512 + nv], lg[:, :nv])

        ctxmgr.close()

    nc.compile()
    return nc


def _host_prep(inputs):
    import ml_dtypes

    bf16 = ml_dtypes.bfloat16
    f32 = np.float32

    x = np.asarray(inputs["x"])
    features = np.asarray(inputs["features"], dtype=f32)
    hidden = np.asarray(inputs["hidden"], dtype=f32)
    emb_table = np.asarray(inputs["emb_table"], dtype=f32)
    Uattn_w = np.asarray(inputs["Uattn_w"], dtype=f32)
    Uattn_b = np.asarray(inputs["Uattn_b"], dtype=f32)
    Wattn_w = np.asarray(inputs["Wattn_w"], dtype=f32)
    Wattn_b = np.asarray(inputs["Wattn_b"], dtype=f32)
    Vattn_w = np.asarray(inputs["Vattn_w"], dtype=f32)
    W_ih = np.asarray(inputs["W_ih"], dtype=f32)
    W_hh = np.asarray(inputs["W_hh"], dtype=f32)
    b_ih = np.asarray(inputs["b_ih"], dtype=f32)
    b_hh = np.asarray(inputs["b_hh"], dtype=f32)
    fc_w = np.asarray(inputs["fc_w"], dtype=f32)
    fc_b = np.asarray(inputs["fc_b"], dtype=f32)
    bn_gamma = np.asarray(inputs["bn_gamma"], dtype=f32)
    bn_beta = np.asarray(inputs["bn_beta"], dtype=f32)
    bn_mean = np.asarray(inputs["bn_mean"], dtype=f32)
    bn_var = np.asarray(inputs["bn_var"], dtype=f32)
    fc2_w = np.asarray(inputs["fc2_w"], dtype=f32)
    fc2_b = np.asarray(inputs["fc2_b"], dtype=f32)

    h = hidden[0]  # [B, U]
    hT16 = np.ascontiguousarray(h.T).astype(bf16)  # [U, B]
    uaT16 = np.ascontiguousarray(Uattn_w.T).astype(bf16)  # [EMB, U]
    waT16 = np.ascontiguousarray(Wattn_w.T).astype(bf16)  # [U, U]
    sbias = np.ascontiguousarray((Uattn_b + Wattn_b).reshape(UC, 128).T)  # [128, UC]
    vT16 = np.ascontiguousarray(Vattn_w[0].reshape(UC, 128).T).astype(bf16)
    e_full = emb_table[x[:, 0].astype(np.int64)]  # [B, EMB] row gather
    eT16 = np.ascontiguousarray(e_full.T).astype(bf16)  # [EMB, B]

    W_ihT = np.ascontiguousarray(W_ih.T)  # [U, 3U]
    W_hhT = np.ascontiguousarray(W_hh.T)
    bsum = (b_ih.astype(np.float64) + b_hh.astype(np.float64))  # [3U]

    bn_s = bn_gamma / np.sqrt(bn_var + BN_EPS)
    bn_t = bn_beta - bn_mean * bn_s
    fc_wTs16 = np.ascontiguousarray(fc_w.T * bn_s[None, :]).astype(bf16)  # [U, U]
    t2_full = (
        fc_b.astype(np.float64) * bn_s.astype(np.float64) + bn_t.astype(np.float64)
    )  # [U]
    t2h_row = t2_full.astype(bf16).reshape(1, U)
    t2l_row = (t2_full - t2h_row[0].astype(np.float64)).astype(bf16).reshape(1, U)

    fc2T = np.zeros((U, N_CORES * VS), dtype=f32)
    fc2T[:, :V] = fc2_w.T
    fc2b_pad = np.zeros(N_CORES * VS, dtype=np.float64)
    fc2b_pad[:V] = fc2_b.astype(np.float64)

    def hilo(v):  # bf16 hi/lo split of a float64 vector -> [1, n] each
        hi = v.astype(bf16)
        lo = (v - hi.astype(np.float64)).astype(bf16)
        return hi.reshape(1, -1), lo.reshape(1, -1)

    in_maps = []
    for c in range(N_CORES):
        bc = slice(c * BC, (c + 1) * BC)
        Sc = slice(c * US, (c + 1) * US)
        feat_c = np.ascontiguousarray(features[bc].reshape(ROWS, EMB))
        m = {
            "feat16": feat_c.astype(bf16),
            "featT16": np.ascontiguousarray(feat_c.T).astype(bf16),
            "uaT16": uaT16,
            "waT16": waT16,
            "hT16": hT16,
            "hTbc16": np.ascontiguousarray(hT16[:, bc]),
            "hU": np.ascontiguousarray(h[:, Sc]),
            "sbias": sbias,
            "vT16": vT16,
            "eT16": eT16,
            "wrzih": np.ascontiguousarray(
                np.hstack([W_ihT[:, Sc], W_ihT[:, U + c * US : U + (c + 1) * US]])
            ).astype(bf16),
            "wrzhh": np.ascontiguousarray(
                np.hstack([W_hhT[:, Sc], W_hhT[:, U + c * US : U + (c + 1) * US]])
            ).astype(bf16),
            "wnih": np.ascontiguousarray(
                W_ihT[:, 2 * U + c * US : 2 * U + (c + 1) * US]
            ).astype(bf16),
            "wnhh": np.ascontiguousarray(
                W_hhT[:, 2 * U + c * US : 2 * U + (c + 1) * US]
            ).astype(bf16),
            "fcw": fc_wTs16,
            "t2h": t2h_row,
            "t2l": t2l_row,
            "fc2t": np.ascontiguousarray(fc2T[:, c * VS : (c + 1) * VS]).astype(
                bf16 if FC2_BF16 else f32
            ),
        }
        brz = np.concatenate([bsum[Sc], bsum[U + c * US : U + (c + 1) * US]])
        m["brz_hi"], m["brz_lo"] = hilo(brz)
        m["bin_hi"], m["bin_lo"] = hilo(
            b_ih[2 * U + c * US : 2 * U + (c + 1) * US].astype(np.float64)
        )
        m["bhn_hi"], m["bhn_lo"] = hilo(
            b_hh[2 * U + c * US : 2 * U + (c + 1) * US].astype(np.float64)
        )
        fb = fc2b_pad[c * VS : (c + 1) * VS]
        if FC2_BF16:
            m["fc2bh"], m["fc2bl"] = hilo(fb)
        else:
            m["fc2b"] = fb.astype(f32).reshape(1, VS)
        in_maps.append(m)
    return in_maps


def kernel(**inputs):
    from concourse.bass_utils import run_bass_kernel_spmd

    if "nc" not in _CACHE:
        _CACHE["nc"] = _build_bass()
    nc = _CACHE["nc"]
    in_maps = _host_prep(inputs)
    res = run_bass_kernel_spmd(nc, in_maps, list(range(N_CORES)))
    rs = res.results
    logits = np.concatenate([rs[c]["logits_c"] for c in range(N_CORES)], axis=1)[
        :, :V
    ]
    h_new = np.concatenate([rs[c]["hnew_c"] for c in range(N_CORES)], axis=1)[
        None, :, :
    ]
    attn = np.concatenate([rs[c]["attn_c"] for c in range(N_CORES)], axis=0)[
        :, :, None
    ]
    return logits.astype(np.float32), h_new.astype(np.float32), attn.astype(
        np.float32
    )


# revision 11
# speedup vs baseline: 1.4832x; 1.1290x over previous
"""TRN2 Bass kernel for nn_Decode (Bahdanau-attention GRU decode step + vocab head).

Sharding across 8 NeuronCores (SPMD, one program, per-core data):
  - Attention: data-parallel over batch (16 rows/core); small attention weights
    replicated (shipped bf16).
  - GRU + fc/BN: tensor-parallel over the hidden dim (128 of 1024 per core),
    stitched with AllGather collectives.
  - fc2 (vocab head) + embedding gather: tensor-parallel over vocab
    (6283 of 50257 rows per core).
  - Host does layout prep only: weight transposes/casts, embedding row gather,
    BN constant folding, output concatenation.

Numerics: bf16 operands for attention/GRU matmuls (≈1e-4 relative error on
outputs), fp32 for the fc/BN/fc2 chain unless FC2_BF16 is set.
"""

import numpy as np

N_CORES = 8
B, L, EMB, U, V = 128, 64, 512, 1024, 50257
BC = B // N_CORES  # 16 batch rows per core
US = U // N_CORES  # 128 hidden slice per core (GRU/fc TP)
VS = -(-V // N_CORES)  # 6283 vocab rows per core
BN_EPS = 1e-3

FC2_BF16 = True  # fc2 weights/matmul in bf16 (halves the dominant DMA)

ROWS = BC * L  # 1024 attention rows per core
RT = ROWS // 128  # 8 row chunks
EC = EMB // 128  # 4 embedding chunks
UC = U // 128  # 8 hidden chunks
NVC = -(-VS // 512)  # 13 vocab column chunks (last is 139 wide)

_CACHE = {}


def _build_bass():
    import concourse.tile as tile
    from concourse import bacc, masks, mybir

    f32 = mybir.dt.float32
    bf16 = mybir.dt.bfloat16
    AF = mybir.ActivationFunctionType
    OP = mybir.AluOpType
    fc2_dt = bf16 if FC2_BF16 else f32

    nc = bacc.Bacc(None, target_bir_lowering=False, num_devices=N_CORES)

    din = lambda n, s, d: nc.dram_tensor(n, s, d, kind="ExternalInput")
    feat16_d = din("feat16", [ROWS, EMB], bf16)  # this core's features, natural
    featT16_d = din("featT16", [EMB, ROWS], bf16)  # transposed
    uaT16_d = din("uaT16", [EMB, U], bf16)  # Uattn_w.T (replicated)
    waT16_d = din("waT16", [U, U], bf16)  # Wattn_w.T (replicated)
    hT16_d = din("hT16", [U, B], bf16)  # full h.T (replicated)
    hTbc16_d = din("hTbc16", [U, BC], bf16)  # this core's batch cols of h.T
    hU_d = din("hU", [B, US], f32)  # this core's hidden-slice of h, natural
    sbias_d = din("sbias", [128, UC], f32)  # (Uattn_b+Wattn_b) col-chunked
    vT16_d = din("vT16", [128, UC], bf16)  # Vattn_w col-chunked
    eT16_d = din("eT16", [EMB, B], bf16)  # gathered embeddings.T (replicated)
    wrzih_d = din("wrzih", [U, 2 * US], bf16)  # W_ih.T r|z slices
    wrzhh_d = din("wrzhh", [U, 2 * US], bf16)
    wnih_d = din("wnih", [U, US], bf16)  # W_ih.T n slice
    wnhh_d = din("wnhh", [U, US], bf16)
    brz_hi_d = din("brz_hi", [1, 2 * US], bf16)  # (b_ih+b_hh) r|z hi/lo
    brz_lo_d = din("brz_lo", [1, 2 * US], bf16)
    bin_hi_d = din("bin_hi", [1, US], bf16)  # b_ih n slice hi/lo
    bin_lo_d = din("bin_lo", [1, US], bf16)
    bhn_hi_d = din("bhn_hi", [1, US], bf16)  # b_hh n slice hi/lo
    bhn_lo_d = din("bhn_lo", [1, US], bf16)
    fcw_d = din("fcw", [U, U], bf16)  # (fc_w.T * bn_scale), full, replicated
    fc2t_d = din("fc2t", [U, VS], fc2_dt)  # fc2_w.T vocab slice
    if FC2_BF16:
        fc2bh_d = din("fc2bh", [1, VS], bf16)
        fc2bl_d = din("fc2bl", [1, VS], bf16)
    else:
        fc2b_d = din("fc2b", [1, VS], f32)

    logits_d = nc.dram_tensor("logits_c", [B, VS], f32, kind="ExternalOutput")
    hnew_d = nc.dram_tensor("hnew_c", [B, US], f32, kind="ExternalOutput")
    attn_d = nc.dram_tensor("attn_c", [BC, L], f32, kind="ExternalOutput")

    groups = [list(range(N_CORES))]

    with tile.TileContext(nc) as tc:
        from contextlib import ExitStack

        ctxmgr = ExitStack()
        consts = ctxmgr.enter_context(tc.tile_pool(name="consts", bufs=1))
        sb_w = ctxmgr.enter_context(tc.tile_pool(name="weights", bufs=1))
        sb_act = ctxmgr.enter_context(tc.tile_pool(name="acts", bufs=1))
        sb_tz = ctxmgr.enter_context(tc.tile_pool(name="tanhz", bufs=3))
        sb_out = ctxmgr.enter_context(tc.tile_pool(name="outs", bufs=3))
        ps_big = ctxmgr.enter_context(tc.tile_pool(name="ps_big", bufs=2, space="PSUM"))
        ps_sc = ctxmgr.enter_context(tc.tile_pool(name="ps_sc", bufs=2, space="PSUM"))
        ps_sm = ps_big
        dram = ctxmgr.enter_context(tc.tile_pool(name="dram", bufs=1, space="DRAM"))
        fc2_pool = ctxmgr.enter_context(tc.tile_pool(name="fc2s", bufs=56))

        ident32 = consts.tile([128, 128], f32)
        masks.make_identity(nc, ident32[:])
        ident16 = consts.tile([128, 128], bf16)
        masks.make_identity(nc, ident16[:])
        ones16 = consts.tile([1, 128], bf16)
        nc.vector.memset(ones16[:], 1.0)
        ones32 = consts.tile([1, 128], f32)
        nc.vector.memset(ones32[:], 1.0)

        # ---------------- load attention inputs ----------------
        featN = []
        for rt in range(RT):
            t = sb_act.tile([128, EMB], bf16, tag=f"featN{rt}")
            nc.sync.dma_start(t[:], feat16_d[rt * 128 : (rt + 1) * 128, :])
            featN.append(t)
        featT = []
        for ec in range(EC):
            t = sb_act.tile([128, ROWS], bf16, tag=f"featT{ec}")
            nc.sync.dma_start(t[:], featT16_d[ec * 128 : (ec + 1) * 128, :])
            featT.append(t)
        uaT = []
        for ec in range(EC):
            t = sb_w.tile([128, U], bf16, tag=f"uaT{ec}")
            nc.sync.dma_start(t[:], uaT16_d[ec * 128 : (ec + 1) * 128, :])
            uaT.append(t)
        waT = []
        for kc in range(UC):
            t = sb_w.tile([128, U], bf16, tag=f"waT{kc}")
            nc.sync.dma_start(t[:], waT16_d[kc * 128 : (kc + 1) * 128, :])
            waT.append(t)
        hTbc = sb_act.tile([128, UC * BC], bf16)  # [128, 8*16] col-chunked
        nc.sync.dma_start(
            hTbc[:].rearrange("p (kc b) -> p kc b", kc=UC),
            hTbc16_d.rearrange("(kc p) b -> p kc b", p=128),
        )
        sbias = consts.tile([128, UC], f32)
        nc.sync.dma_start(sbias[:], sbias_d[:, :])
        vT16 = consts.tile([128, UC], bf16)
        nc.sync.dma_start(vT16[:], vT16_d[:, :])

        # ---------------- early streaming loads (pure, dep-free) ----------------
        hT_full = []  # bf16 [128, 128] chunks of h^T for the gh matmuls
        for kc in range(UC):
            t = sb_act.tile([128, B], bf16, tag=f"hTf{kc}")
            nc.sync.dma_start(t[:], hT16_d[kc * 128 : (kc + 1) * 128, :])
            hT_full.append(t)

        wrzih = sb_w.tile([128, UC * 2 * US], bf16)
        nc.sync.dma_start(
            wrzih[:].rearrange("p (kc n) -> p kc n", kc=UC),
            wrzih_d.rearrange("(kc p) n -> p kc n", p=128),
        )
        wrzhh = sb_w.tile([128, UC * 2 * US], bf16)
        nc.sync.dma_start(
            wrzhh[:].rearrange("p (kc n) -> p kc n", kc=UC),
            wrzhh_d.rearrange("(kc p) n -> p kc n", p=128),
        )
        wnih = sb_w.tile([128, UC * US], bf16)
        nc.sync.dma_start(
            wnih[:].rearrange("p (kc n) -> p kc n", kc=UC),
            wnih_d.rearrange("(kc p) n -> p kc n", p=128),
        )
        wnhh = sb_w.tile([128, UC * US], bf16)
        nc.sync.dma_start(
            wnhh[:].rearrange("p (kc n) -> p kc n", kc=UC),
            wnhh_d.rearrange("(kc p) n -> p kc n", p=128),
        )
        brz_hi = sb_w.tile([1, 2 * US], bf16)
        nc.sync.dma_start(brz_hi[:], brz_hi_d[:, :])
        brz_lo = sb_w.tile([1, 2 * US], bf16)
        nc.sync.dma_start(brz_lo[:], brz_lo_d[:, :])
        bin_hi = sb_w.tile([1, US], bf16)
        nc.sync.dma_start(bin_hi[:], bin_hi_d[:, :])
        bin_lo = sb_w.tile([1, US], bf16)
        nc.sync.dma_start(bin_lo[:], bin_lo_d[:, :])
        bhn_hi = sb_w.tile([1, US], bf16)
        nc.sync.dma_start(bhn_hi[:], bhn_hi_d[:, :])
        bhn_lo = sb_w.tile([1, US], bf16)
        nc.sync.dma_start(bhn_lo[:], bhn_lo_d[:, :])

        fcw = sb_w.tile([128, UC * U], bf16)
        nc.sync.dma_start(
            fcw[:].rearrange("p (kc n) -> p kc n", kc=UC),
            fcw_d.rearrange("(kc p) n -> p kc n", p=128),
        )
        if FC2_BF16:
            fc2bh = sb_w.tile([1, VS], bf16)
            nc.sync.dma_start(fc2bh[:], fc2bh_d[:, :])
            fc2bl = sb_w.tile([1, VS], bf16)
            nc.sync.dma_start(fc2bl[:], fc2bl_d[:, :])
        else:
            fc2b = sb_w.tile([1, VS], f32)
            nc.sync.dma_start(fc2b[:], fc2b_d[:, :])

        # fc2 weight stream: prefetched tiles, consumed in order by the matmuls
        fc2_wt = {}
        for vc in range(NVC):
            nv = min(512, VS - vc * 512)
            for kc in range(UC):
                wt = fc2_pool.tile([128, 512], fc2_dt, tag="fc2w", name=f"wt{vc}_{kc}")
                nc.sync.dma_start(
                    wt[:, :nv],
                    fc2t_d[kc * 128 : (kc + 1) * 128, vc * 512 : vc * 512 + nv],
                )
                fc2_wt[(vc, kc)] = wt

        # ---------------- Wh^T [u, b_loc] ----------------
        # Wh = h_c @ Wattn.T as [16, 1024], then PE-transpose to [u, 16].
        wh_sb = sb_act.tile([BC, U], f32)
        for half in range(2):
            ps = ps_big.tile([128, 512], f32, tag="big", name="ps_wh")[:BC, :]
            for kc in range(UC):
                nc.tensor.matmul(
                    ps[:],
                    hTbc[:, kc * BC : (kc + 1) * BC],
                    waT[kc][:, half * 512 : (half + 1) * 512],
                    start=(kc == 0),
                    stop=(kc == UC - 1),
                )
            nc.scalar.activation(
                wh_sb[:, half * 512 : (half + 1) * 512], ps[:], AF.Copy
            )
        whT = sb_act.tile([128, UC * BC], f32)  # [128, 8*16] col-chunked by uc
        for kc in range(UC):
            pst = ps_sm.tile([128, 128], f32, tag="tp", name="ps_whT")[:, :BC]
            nc.tensor.matmul(
                pst[:],
                wh_sb[:, kc * 128 : (kc + 1) * 128],
                ident32[:BC, :BC],
                is_transpose=True,
            )
            nc.vector.tensor_copy(whT[:, kc * BC : (kc + 1) * BC], pst[:])

        # ---------------- Uf + tanh + score ----------------
        # psum_uf[u_chunk, 512 rows] = featT.T @ uaT chunks; add Wh^T broadcast
        # over l; tanh via ACT (bias = Uattn_b+Wattn_b per-partition);
        # score accumulates v^T @ tanhz over u chunks.
        ps_score = []
        for rg in range(2):
            ps_s = ps_sc.tile([1, 512], f32, tag="sc")
            ps_score.append(ps_s)
            for uc in range(UC):
                ps_uf = ps_big.tile([128, 512], f32, tag="big")
                for ec in range(EC):
                    nc.tensor.matmul(
                        ps_uf[:],
                        uaT[ec][:, uc * 128 : (uc + 1) * 128],
                        featT[ec][:, rg * 512 : (rg + 1) * 512],
                        start=(ec == 0),
                        stop=(ec == EC - 1),
                    )
                # add Wh^T: row rg*512+i has b = (rg*512+i)//64
                wh_bcast = (
                    whT[:, uc * BC + rg * 8 : uc * BC + rg * 8 + 8]
                    .unsqueeze(2)
                    .to_broadcast([128, 8, 64])
                )
                nc.vector.tensor_tensor(
                    out=ps_uf[:].rearrange("p (b l) -> p b l", l=64),
                    in0=ps_uf[:].rearrange("p (b l) -> p b l", l=64),
                    in1=wh_bcast,
                    op=OP.add,
                )
                tz = sb_tz.tile([128, 512], bf16, tag="tanhz")
                nc.scalar.activation(
                    tz[:], ps_uf[:], AF.Tanh, bias=sbias[:, uc : uc + 1]
                )
                nc.tensor.matmul(
                    ps_s[:],
                    vT16[:, uc : uc + 1],
                    tz[:],
                    start=(uc == 0),
                    stop=(uc == UC - 1),
                )

        # ---------------- softmax over l (per b) ----------------
        score_sb = sb_act.tile([1, ROWS], f32)
        for rg in range(2):
            nc.scalar.activation(
                score_sb[:, rg * 512 : (rg + 1) * 512], ps_score[rg][:], AF.Copy
            )
        sc3 = score_sb[:].rearrange("p (b l) -> p b l", l=64)
        mx = sb_act.tile([1, BC], f32)
        nc.vector.reduce_max(mx[:], sc3, axis=mybir.AxisListType.X)
        mx_b = mx[:].unsqueeze(2).to_broadcast([1, BC, 64])
        nc.vector.tensor_tensor(out=sc3, in0=sc3, in1=mx_b, op=OP.subtract)
        esb = sb_act.tile([1, ROWS], f32)
        nc.scalar.activation(esb[:], score_sb[:], AF.Exp)
        ssum = sb_act.tile([1, BC], f32)
        nc.vector.reduce_sum(
            ssum[:], esb[:].rearrange("p (b l) -> p b l", l=64), axis=mybir.AxisListType.X
        )
        rsum = sb_act.tile([1, BC], f32)
        nc.vector.reciprocal(rsum[:], ssum[:])
        attn_sb = sb_act.tile([1, ROWS], f32)
        nc.vector.tensor_tensor(
            out=attn_sb[:].rearrange("p (b l) -> p b l", l=64),
            in0=esb[:].rearrange("p (b l) -> p b l", l=64),
            in1=rsum[:].unsqueeze(2).to_broadcast([1, BC, 64]),
            op=OP.mult,
        )
        nc.scalar.dma_start(attn_d[:, :], attn_sb[:])
        attn16 = sb_act.tile([1, ROWS], bf16)
        nc.vector.tensor_copy(attn16[:], attn_sb[:])

        # ---------------- ctx = attn-weighted sum of features ----------------
        # Build block "diagonal" lhsT tiles: Ablk[:, rt*16+b] holds attn[b, :]
        # at partitions (b-2rt)*64..+64 for b in {2rt, 2rt+1}, zeros elsewhere.
        ablk = sb_act.tile([128, RT * BC], bf16)
        nc.vector.memset(ablk[:], 0.0)
        for rt in range(RT):
            pst = ps_sm.tile([128, 256], bf16, tag="tp", name="ps_attnT")[:, :1]
            nc.tensor.matmul(
                pst[:],
                attn16[:, rt * 128 : (rt + 1) * 128],
                ident16[:1, :1],
                is_transpose=True,
            )
            b0 = 2 * rt
            nc.vector.tensor_copy(
                ablk[0:64, rt * BC + b0 : rt * BC + b0 + 1], pst[0:64, :]
            )
            nc.vector.tensor_copy(
                ablk[64:128, rt * BC + b0 + 1 : rt * BC + b0 + 2], pst[64:128, :]
            )
        ps_ctx = ps_big.tile([128, 512], f32, tag="big", name="ps_ctx")[:BC, :EMB]
        for rt in range(RT):
            nc.tensor.matmul(
                ps_ctx[:],
                ablk[:, rt * BC : (rt + 1) * BC],
                featN[rt][:],
                start=(rt == 0),
                stop=(rt == RT - 1),
            )
        ctx_sb = sb_act.tile([BC, EMB], bf16)
        nc.scalar.activation(ctx_sb[:], ps_ctx[:], AF.Copy)

        # ---------------- AllGather ctx -> full batch (bf16) ----------------
        ctx_bounce = dram.tile([BC, EMB], bf16)
        ctx_ag = dram.tile([B, EMB], bf16)
        nc.scalar.dma_start(ctx_bounce[:], ctx_sb[:])
        nc.gpsimd.collective_compute(
            "AllGather",
            OP.bypass,
            replica_groups=groups,
            ins=[ctx_bounce.opt()],
            outs=[ctx_ag.opt()],
        )

        # ---------------- ginT (bf16): ctx^T chunks + e^T chunks ----------------
        ginT = sb_act.tile([128, UC * B], bf16)  # col-chunked [kc][128, 128]
        cg_sb = sb_act.tile([128, EMB], bf16)
        nc.scalar.dma_start(cg_sb[:], ctx_ag[:, :])
        for ec in range(EC):
            pst = ps_sm.tile([128, 128], bf16, tag="tp", name="pst_gin")
            nc.tensor.matmul(
                pst[:],
                cg_sb[:, ec * 128 : (ec + 1) * 128],
                ident16[:, :],
                is_transpose=True,
            )
            nc.vector.tensor_copy(ginT[:, ec * B : (ec + 1) * B], pst[:])
        nc.sync.dma_start(
            ginT[:, EC * B :].rearrange("p (kc b) -> p kc b", kc=EC),
            eT16_d.rearrange("(kc p) b -> p kc b", p=128),
        )
        # ---------------- GRU gates (TP slice of 128 u per core) ----------------
        ps_rz = ps_big.tile([128, 512], f32, tag="big", name="ps_rz")[:, : 2 * US]
        n_mm = 2 * UC + 2
        i_mm = 0
        for kc in range(UC):
            nc.tensor.matmul(
                ps_rz[:],
                hT_full[kc][:],
                wrzhh[:, kc * 2 * US : (kc + 1) * 2 * US],
                start=(i_mm == 0),
                stop=(i_mm == n_mm - 1),
            )
            i_mm += 1
        for kc in range(UC):
            nc.tensor.matmul(
                ps_rz[:],
                ginT[:, kc * B : (kc + 1) * B],
                wrzih[:, kc * 2 * US : (kc + 1) * 2 * US],
                start=(i_mm == 0),
                stop=(i_mm == n_mm - 1),
            )
            i_mm += 1
        nc.tensor.matmul(ps_rz[:], ones16[:], brz_hi[:], start=False, stop=False)
        nc.tensor.matmul(ps_rz[:], ones16[:], brz_lo[:], start=False, stop=True)

        ps_hn = ps_sm.tile([128, 128], f32, tag="tp")
        for kc in range(UC):
            nc.tensor.matmul(
                ps_hn[:],
                hT_full[kc][:],
                wnhh[:, kc * US : (kc + 1) * US],
                start=(kc == 0),
                stop=False,
            )
        nc.tensor.matmul(ps_hn[:], ones16[:], bhn_hi[:], start=False, stop=False)
        nc.tensor.matmul(ps_hn[:], ones16[:], bhn_lo[:], start=False, stop=True)

        ps_in = ps_sm.tile([128, 128], f32, tag="tp")
        for kc in range(UC):
            nc.tensor.matmul(
                ps_in[:],
                ginT[:, kc * B : (kc + 1) * B],
                wnih[:, kc * US : (kc + 1) * US],
                start=(kc == 0),
                stop=False,
            )
        nc.tensor.matmul(ps_in[:], ones16[:], bin_hi[:], start=False, stop=False)
        nc.tensor.matmul(ps_in[:], ones16[:], bin_lo[:], start=False, stop=True)

        r_sb = sb_act.tile([B, US], f32)
        nc.scalar.activation(r_sb[:], ps_rz[:, 0:US], AF.Sigmoid)
        z_sb = sb_act.tile([B, US], f32)
        nc.scalar.activation(z_sb[:], ps_rz[:, US : 2 * US], AF.Sigmoid)
        rhn = sb_act.tile([B, US], f32)
        nc.vector.tensor_tensor(out=rhn[:], in0=ps_hn[:], in1=r_sb[:], op=OP.mult)
        nc.vector.tensor_tensor(out=ps_in[:], in0=ps_in[:], in1=rhn[:], op=OP.add)
        n_sb = sb_act.tile([B, US], f32)
        nc.scalar.activation(n_sb[:], ps_in[:], AF.Tanh)
        hU = sb_act.tile([B, US], f32)
        nc.scalar.dma_start(hU[:], hU_d[:, :])
        d_sb = sb_act.tile([B, US], f32)
        nc.vector.tensor_tensor(out=d_sb[:], in0=hU[:], in1=n_sb[:], op=OP.subtract)
        zd_sb = sb_act.tile([B, US], f32)
        nc.vector.tensor_tensor(out=zd_sb[:], in0=z_sb[:], in1=d_sb[:], op=OP.mult)
        hnew_sb = sb_act.tile([B, US], f32)
        nc.vector.tensor_tensor(out=hnew_sb[:], in0=n_sb[:], in1=zd_sb[:], op=OP.add)
        nc.scalar.dma_start(hnew_d[:, :], hnew_sb[:])

        # ---------------- AllGather h_new ----------------
        hnew_bounce = dram.tile([B, US], f32)
        hnew_ag = dram.tile([N_CORES * B, US], f32)  # blocks [c][b, u_loc]
        nc.scalar.dma_start(hnew_bounce[:], hnew_sb[:])
        nc.gpsimd.collective_compute(
            "AllGather",
            OP.bypass,
            replica_groups=groups,
            ins=[hnew_bounce.opt()],
            outs=[hnew_ag.opt()],
        )

        # transpose blocks to h_new^T [u, b] chunks (f32 for the fc matmul)
        hnT = sb_act.tile([128, UC * B], bf16)
        for c2 in range(N_CORES):
            blk = sb_tz.tile([128, US], f32, tag="hn_blk")
            nc.scalar.dma_start(blk[:], hnew_ag[c2 * B : (c2 + 1) * B, :])
            pst = ps_sm.tile([128, 128], f32, tag="tp")
            nc.tensor.matmul(pst[:], blk[:], ident32[:, :], is_transpose=True)
            nc.vector.tensor_copy(hnT[:, c2 * B : (c2 + 1) * B], pst[:])

        # ---------------- y = h_new @ fc_w^T (BN folded), replicated ----------------
        ybn_sb = sb_act.tile([B, U], f32)
        for half in range(2):
            ps_y = ps_big.tile([128, 512], f32, tag="big", name="ps_y")
            for kc in range(UC):
                nc.tensor.matmul(
                    ps_y[:],
                    hnT[:, kc * B : (kc + 1) * B],
                    fcw[:, kc * U + half * 512 : kc * U + (half + 1) * 512],
                    start=(kc == 0),
                    stop=(kc == UC - 1),
                )
            nc.vector.tensor_copy(ybn_sb[:, half * 512 : (half + 1) * 512], ps_y[:])

        fc2_in_dt = bf16 if FC2_BF16 else f32
        ybnT = sb_act.tile([128, UC * B], fc2_in_dt)
        for c2 in range(N_CORES):
            pst = ps_sm.tile([128, 128], f32, tag="tp")
            nc.tensor.matmul(
                pst[:], ybn_sb[:, c2 * US : (c2 + 1) * US], ident32[:, :],
                is_transpose=True,
            )
            nc.vector.tensor_copy(ybnT[:, c2 * B : (c2 + 1) * B], pst[:])

        # ---------------- fc2: logits = ybn @ fc2_w^T + fc2_b ----------------
        for vc in range(NVC):
            nv = min(512, VS - vc * 512)
            ps_l = ps_big.tile([128, 512], f32, tag="big")
            for kc in range(UC):
                wt = fc2_wt[(vc, kc)]
                nc.tensor.matmul(
                    ps_l[:, :nv],
                    ybnT[:, kc * B : (kc + 1) * B],
                    wt[:, :nv],
                    start=(kc == 0),
                    stop=False,
                )
            if FC2_BF16:
                nc.tensor.matmul(
                    ps_l[:, :nv],
                    ones16[:],
                    fc2bh[:, vc * 512 : vc * 512 + nv],
                    start=False,
                    stop=False,
                )
                nc.tensor.matmul(
                    ps_l[:, :nv],
                    ones16[:],
                    fc2bl[:, vc * 512 : vc * 512 + nv],
                    start=False,
                    stop=True,
                )
            else:
                nc.tensor.matmul(
                    ps_l[:, :nv],
                    ones32[:],
                    fc2b[:, vc * 512 : vc * 512 + nv],
                    start=False,
                    stop=True,
                )
            lg = sb_out.tile([B, 512], f32, tag="lg")
            nc.vector.tensor_copy(lg[:, :nv], ps_l[:, :nv])
            nc.scalar.dma_start(logits_d[:, vc * 512 : vc * 512 + nv], lg[:, :nv])

        ctxmgr.close()

    nc.compile()
    return nc


def _host_prep(inputs):
    import ml_dtypes

    bf16 = ml_dtypes.bfloat16
    f32 = np.float32

    x = np.asarray(inputs["x"])
    features = np.asarray(inputs["features"], dtype=f32)
    hidden = np.asarray(inputs["hidden"], dtype=f32)
    emb_table = np.asarray(inputs["emb_table"], dtype=f32)
    Uattn_w = np.asarray(inputs["Uattn_w"], dtype=f32)
    Uattn_b = np.asarray(inputs["Uattn_b"], dtype=f32)
    Wattn_w = np.asarray(inputs["Wattn_w"], dtype=f32)
    Wattn_b = np.asarray(inputs["Wattn_b"], dtype=f32)
    Vattn_w = np.asarray(inputs["Vattn_w"], dtype=f32)
    W_ih = np.asarray(inputs["W_ih"], dtype=f32)
    W_hh = np.asarray(inputs["W_hh"], dtype=f32)
    b_ih = np.asarray(inputs["b_ih"], dtype=f32)
    b_hh = np.asarray(inputs["b_hh"], dtype=f32)
    fc_w = np.asarray(inputs["fc_w"], dtype=f32)
    fc_b = np.asarray(inputs["fc_b"], dtype=f32)
    bn_gamma = np.asarray(inputs["bn_gamma"], dtype=f32)
    bn_beta = np.asarray(inputs["bn_beta"], dtype=f32)
    bn_mean = np.asarray(inputs["bn_mean"], dtype=f32)
    bn_var = np.asarray(inputs["bn_var"], dtype=f32)
    fc2_w = np.asarray(inputs["fc2_w"], dtype=f32)
    fc2_b = np.asarray(inputs["fc2_b"], dtype=f32)

    h = hidden[0]  # [B, U]
    hT16 = np.ascontiguousarray(h.T).astype(bf16)  # [U, B]
    uaT16 = np.ascontiguousarray(Uattn_w.T).astype(bf16)  # [EMB, U]
    waT16 = np.ascontiguousarray(Wattn_w.T).astype(bf16)  # [U, U]
    sbias = np.ascontiguousarray((Uattn_b + Wattn_b).reshape(UC, 128).T)  # [128, UC]
    vT16 = np.ascontiguousarray(Vattn_w[0].reshape(UC, 128).T).astype(bf16)
    e_full = emb_table[x[:, 0].astype(np.int64)]  # [B, EMB] row gather
    eT16 = np.ascontiguousarray(e_full.T).astype(bf16)  # [EMB, B]

    W_ihT = np.ascontiguousarray(W_ih.T)  # [U, 3U]
    W_hhT = np.ascontiguousarray(W_hh.T)
    bsum = (b_ih.astype(np.float64) + b_hh.astype(np.float64))  # [3U]

    bn_s = bn_gamma / np.sqrt(bn_var + BN_EPS)
    bn_t = bn_beta - bn_mean * bn_s
    fc_wTs16 = np.ascontiguousarray(fc_w.T * bn_s[None, :]).astype(bf16)  # [U, U]
    t2_full = (
        fc_b.astype(np.float64) * bn_s.astype(np.float64) + bn_t.astype(np.float64)
    )  # [U]

    fc2T = np.zeros((U, N_CORES * VS), dtype=f32)
    fc2T[:, :V] = fc2_w.T
    fc2b_pad = np.zeros(N_CORES * VS, dtype=np.float64)
    fc2b_pad[:V] = fc2_b.astype(np.float64) + t2_full @ fc2_w.T.astype(np.float64)

    def hilo(v):  # bf16 hi/lo split of a float64 vector -> [1, n] each
        hi = v.astype(bf16)
        lo = (v - hi.astype(np.float64)).astype(bf16)
        return hi.reshape(1, -1), lo.reshape(1, -1)

    in_maps = []
    for c in range(N_CORES):
        bc = slice(c * BC, (c + 1) * BC)
        Sc = slice(c * US, (c + 1) * US)
        feat_c = np.ascontiguousarray(features[bc].reshape(ROWS, EMB))
        m = {
            "feat16": feat_c.astype(bf16),
            "featT16": np.ascontiguousarray(feat_c.T).astype(bf16),
            "uaT16": uaT16,
            "waT16": waT16,
            "hT16": hT16,
            "hTbc16": np.ascontiguousarray(hT16[:, bc]),
            "hU": np.ascontiguousarray(h[:, Sc]),
            "sbias": sbias,
            "vT16": vT16,
            "eT16": eT16,
            "wrzih": np.ascontiguousarray(
                np.hstack([W_ihT[:, Sc], W_ihT[:, U + c * US : U + (c + 1) * US]])
            ).astype(bf16),
            "wrzhh": np.ascontiguousarray(
                np.hstack([W_hhT[:, Sc], W_hhT[:, U + c * US : U + (c + 1) * US]])
            ).astype(bf16),
            "wnih": np.ascontiguousarray(
                W_ihT[:, 2 * U + c * US : 2 * U + (c + 1) * US]
            ).astype(bf16),
            "wnhh": np.ascontiguousarray(
                W_hhT[:, 2 * U + c * US : 2 * U + (c + 1) * US]
            ).astype(bf16),
            "fcw": fc_wTs16,
            "fc2t": np.ascontiguousarray(fc2T[:, c * VS : (c + 1) * VS]).astype(
                bf16 if FC2_BF16 else f32
            ),
        }
        brz = np.concatenate([bsum[Sc], bsum[U + c * US : U + (c + 1) * US]])
        m["brz_hi"], m["brz_lo"] = hilo(brz)
        m["bin_hi"], m["bin_lo"] = hilo(
            b_ih[2 * U + c * US : 2 * U + (c + 1) * US].astype(np.float64)
        )
        m["bhn_hi"], m["bhn_lo"] = hilo(
            b_hh[2 * U + c * US : 2 * U + (c + 1) * US].astype(np.float64)
        )
        fb = fc2b_pad[c * VS : (c + 1) * VS]
        if FC2_BF16:
            m["fc2bh"], m["fc2bl"] = hilo(fb)
        else:
            m["fc2b"] = fb.astype(f32).reshape(1, VS)
        in_maps.append(m)
    return in_maps


def kernel(**inputs):
    from concourse.bass_utils import run_bass_kernel_spmd

    if "nc" not in _CACHE:
        _CACHE["nc"] = _build_bass()
    nc = _CACHE["nc"]
    in_maps = _host_prep(inputs)
    res = run_bass_kernel_spmd(nc, in_maps, list(range(N_CORES)))
    rs = res.results
    logits = np.concatenate([rs[c]["logits_c"] for c in range(N_CORES)], axis=1)[
        :, :V
    ]
    h_new = np.concatenate([rs[c]["hnew_c"] for c in range(N_CORES)], axis=1)[
        None, :, :
    ]
    attn = np.concatenate([rs[c]["attn_c"] for c in range(N_CORES)], axis=0)[
        :, :, None
    ]
    return logits.astype(np.float32), h_new.astype(np.float32), attn.astype(
        np.float32
    )


# revision 12
# speedup vs baseline: 1.6177x; 1.0907x over previous
"""TRN2 Bass kernel for nn_Decode (Bahdanau-attention GRU decode step + vocab head).

Sharding across 8 NeuronCores (SPMD, one program, per-core data):
  - Attention: data-parallel over batch (16 rows/core); small attention weights
    replicated (shipped bf16).
  - GRU + fc/BN: tensor-parallel over the hidden dim (128 of 1024 per core),
    stitched with AllGather collectives.
  - fc2 (vocab head) + embedding gather: tensor-parallel over vocab
    (6283 of 50257 rows per core).
  - Host does layout prep only: weight transposes/casts, embedding row gather,
    BN constant folding, output concatenation.

Numerics: bf16 operands for attention/GRU matmuls (≈1e-4 relative error on
outputs), fp32 for the fc/BN/fc2 chain unless FC2_BF16 is set.
"""

import numpy as np

N_CORES = 8
B, L, EMB, U, V = 128, 64, 512, 1024, 50257
BC = B // N_CORES  # 16 batch rows per core
US = U // N_CORES  # 128 hidden slice per core (GRU/fc TP)
VS = -(-V // N_CORES)  # 6283 vocab rows per core
BN_EPS = 1e-3

FC2_BF16 = True  # fc2 weights/matmul in bf16 (halves the dominant DMA)

ROWS = BC * L  # 1024 attention rows per core
RT = ROWS // 128  # 8 row chunks
EC = EMB // 128  # 4 embedding chunks
UC = U // 128  # 8 hidden chunks
NVC = -(-VS // 512)  # 13 vocab column chunks (last is 139 wide)

_CACHE = {}


def _build_bass():
    import concourse.tile as tile
    from concourse import bacc, masks, mybir

    f32 = mybir.dt.float32
    bf16 = mybir.dt.bfloat16
    AF = mybir.ActivationFunctionType
    OP = mybir.AluOpType
    fc2_dt = bf16 if FC2_BF16 else f32

    nc = bacc.Bacc(None, target_bir_lowering=False, num_devices=N_CORES)

    din = lambda n, s, d: nc.dram_tensor(n, s, d, kind="ExternalInput")
    feat16_d = din("feat16", [ROWS, EMB], bf16)  # this core's features, natural
    featT16_d = din("featT16", [EMB, ROWS], bf16)  # transposed
    uaT16_d = din("uaT16", [EMB, U], bf16)  # Uattn_w.T (replicated)
    waT16_d = din("waT16", [U, U], bf16)  # Wattn_w.T (replicated)
    hT16_d = din("hT16", [U, B], bf16)  # full h.T (replicated)
    hTbc16_d = din("hTbc16", [U, BC], bf16)  # this core's batch cols of h.T
    hU_d = din("hU", [B, US], f32)  # this core's hidden-slice of h, natural
    sbias_d = din("sbias", [128, UC], f32)  # (Uattn_b+Wattn_b) col-chunked
    vT16_d = din("vT16", [128, UC], bf16)  # Vattn_w col-chunked
    eT16_d = din("eT16", [EMB, B], bf16)  # gathered embeddings.T (replicated)
    wrzih_d = din("wrzih", [U, 2 * US], bf16)  # W_ih.T r|z slices
    wrzhh_d = din("wrzhh", [U, 2 * US], bf16)
    wnih_d = din("wnih", [U, US], bf16)  # W_ih.T n slice
    wnhh_d = din("wnhh", [U, US], bf16)
    brz_hi_d = din("brz_hi", [1, 2 * US], bf16)  # (b_ih+b_hh) r|z hi/lo
    brz_lo_d = din("brz_lo", [1, 2 * US], bf16)
    bin_hi_d = din("bin_hi", [1, US], bf16)  # b_ih n slice hi/lo
    bin_lo_d = din("bin_lo", [1, US], bf16)
    bhn_hi_d = din("bhn_hi", [1, US], bf16)  # b_hh n slice hi/lo
    bhn_lo_d = din("bhn_lo", [1, US], bf16)
    fcw_d = din("fcw", [U, U], bf16)  # (fc_w.T * bn_scale), full, replicated
    fc2t_d = din("fc2t", [U, VS], fc2_dt)  # fc2_w.T vocab slice
    if FC2_BF16:
        fc2bh_d = din("fc2bh", [1, VS], bf16)
        fc2bl_d = din("fc2bl", [1, VS], bf16)
    else:
        fc2b_d = din("fc2b", [1, VS], f32)

    logits_d = nc.dram_tensor("logits_c", [B, VS], f32, kind="ExternalOutput")
    hnew_d = nc.dram_tensor("hnew_c", [B, US], f32, kind="ExternalOutput")
    attn_d = nc.dram_tensor("attn_c", [BC, L], f32, kind="ExternalOutput")

    groups = [list(range(N_CORES))]

    with tile.TileContext(nc) as tc:
        from contextlib import ExitStack

        ctxmgr = ExitStack()
        consts = ctxmgr.enter_context(tc.tile_pool(name="consts", bufs=1))
        sb_w = ctxmgr.enter_context(tc.tile_pool(name="weights", bufs=1))
        sb_act = ctxmgr.enter_context(tc.tile_pool(name="acts", bufs=1))
        sb_tz = ctxmgr.enter_context(tc.tile_pool(name="tanhz", bufs=3))
        sb_out = ctxmgr.enter_context(tc.tile_pool(name="outs", bufs=3))
        ps_big = ctxmgr.enter_context(tc.tile_pool(name="ps_big", bufs=2, space="PSUM"))
        ps_sc = ctxmgr.enter_context(tc.tile_pool(name="ps_sc", bufs=2, space="PSUM"))
        ps_sm = ps_big
        dram = ctxmgr.enter_context(tc.tile_pool(name="dram", bufs=1, space="DRAM"))
        fc2_pool = ctxmgr.enter_context(tc.tile_pool(name="fc2s", bufs=56))

        ident32 = consts.tile([128, 128], f32)
        masks.make_identity(nc, ident32[:])
        ident16 = consts.tile([128, 128], bf16)
        masks.make_identity(nc, ident16[:])
        ones16 = consts.tile([1, 128], bf16)
        nc.vector.memset(ones16[:], 1.0)
        ones32 = consts.tile([1, 128], f32)
        nc.vector.memset(ones32[:], 1.0)

        # ---------------- core-alignment dummy collective ----------------
        align_in = dram.tile([1, 8], f32)
        align_out = dram.tile([N_CORES, 8], f32)
        nc.gpsimd.dma_start(align_in[:], ones32[:, 0:8])
        nc.gpsimd.collective_compute(
            "AllGather",
            OP.bypass,
            replica_groups=groups,
            ins=[align_in.opt()],
            outs=[align_out.opt()],
        )

        # ---------------- load attention inputs ----------------
        featT = []
        for ec in range(EC):
            t = sb_act.tile([128, ROWS], bf16, tag=f"featT{ec}")
            nc.sync.dma_start(t[:], featT16_d[ec * 128 : (ec + 1) * 128, :])
            featT.append(t)
        uaT = []
        for ec in range(EC):
            t = sb_w.tile([128, U], bf16, tag=f"uaT{ec}")
            nc.sync.dma_start(t[:], uaT16_d[ec * 128 : (ec + 1) * 128, :])
            uaT.append(t)
        featN = []
        for rt in range(RT):
            t = sb_act.tile([128, EMB], bf16, tag=f"featN{rt}")
            nc.sync.dma_start(t[:], feat16_d[rt * 128 : (rt + 1) * 128, :])
            featN.append(t)
        waT = []
        for kc in range(UC):
            t = sb_w.tile([128, U], bf16, tag=f"waT{kc}")
            nc.sync.dma_start(t[:], waT16_d[kc * 128 : (kc + 1) * 128, :])
            waT.append(t)
        hTbc = sb_act.tile([128, UC * BC], bf16)  # [128, 8*16] col-chunked
        nc.sync.dma_start(
            hTbc[:].rearrange("p (kc b) -> p kc b", kc=UC),
            hTbc16_d.rearrange("(kc p) b -> p kc b", p=128),
        )
        sbias = consts.tile([128, UC], f32)
        nc.sync.dma_start(sbias[:], sbias_d[:, :])
        vT16 = consts.tile([128, UC], bf16)
        nc.sync.dma_start(vT16[:], vT16_d[:, :])

        # ---------------- early streaming loads (pure, dep-free) ----------------
        hT_full = []  # bf16 [128, 128] chunks of h^T for the gh matmuls
        for kc in range(UC):
            t = sb_act.tile([128, B], bf16, tag=f"hTf{kc}")
            nc.sync.dma_start(t[:], hT16_d[kc * 128 : (kc + 1) * 128, :])
            hT_full.append(t)

        wrzih = sb_w.tile([128, UC * 2 * US], bf16)
        nc.sync.dma_start(
            wrzih[:].rearrange("p (kc n) -> p kc n", kc=UC),
            wrzih_d.rearrange("(kc p) n -> p kc n", p=128),
        )
        wrzhh = sb_w.tile([128, UC * 2 * US], bf16)
        nc.sync.dma_start(
            wrzhh[:].rearrange("p (kc n) -> p kc n", kc=UC),
            wrzhh_d.rearrange("(kc p) n -> p kc n", p=128),
        )
        wnih = sb_w.tile([128, UC * US], bf16)
        nc.sync.dma_start(
            wnih[:].rearrange("p (kc n) -> p kc n", kc=UC),
            wnih_d.rearrange("(kc p) n -> p kc n", p=128),
        )
        wnhh = sb_w.tile([128, UC * US], bf16)
        nc.sync.dma_start(
            wnhh[:].rearrange("p (kc n) -> p kc n", kc=UC),
            wnhh_d.rearrange("(kc p) n -> p kc n", p=128),
        )
        brz_hi = sb_w.tile([1, 2 * US], bf16)
        nc.sync.dma_start(brz_hi[:], brz_hi_d[:, :])
        brz_lo = sb_w.tile([1, 2 * US], bf16)
        nc.sync.dma_start(brz_lo[:], brz_lo_d[:, :])
        bin_hi = sb_w.tile([1, US], bf16)
        nc.sync.dma_start(bin_hi[:], bin_hi_d[:, :])
        bin_lo = sb_w.tile([1, US], bf16)
        nc.sync.dma_start(bin_lo[:], bin_lo_d[:, :])
        bhn_hi = sb_w.tile([1, US], bf16)
        nc.sync.dma_start(bhn_hi[:], bhn_hi_d[:, :])
        bhn_lo = sb_w.tile([1, US], bf16)
        nc.sync.dma_start(bhn_lo[:], bhn_lo_d[:, :])

        fcw = sb_w.tile([128, UC * U], bf16)
        nc.sync.dma_start(
            fcw[:].rearrange("p (kc n) -> p kc n", kc=UC),
            fcw_d.rearrange("(kc p) n -> p kc n", p=128),
        )
        if FC2_BF16:
            fc2bh = sb_w.tile([1, VS], bf16)
            nc.sync.dma_start(fc2bh[:], fc2bh_d[:, :])
            fc2bl = sb_w.tile([1, VS], bf16)
            nc.sync.dma_start(fc2bl[:], fc2bl_d[:, :])
        else:
            fc2b = sb_w.tile([1, VS], f32)
            nc.sync.dma_start(fc2b[:], fc2b_d[:, :])

        # fc2 weight stream: prefetched tiles, consumed in order by the matmuls
        fc2_wt = {}
        for vc in range(NVC):
            nv = min(512, VS - vc * 512)
            for kc in range(UC):
                wt = fc2_pool.tile([128, 512], fc2_dt, tag="fc2w", name=f"wt{vc}_{kc}")
                nc.sync.dma_start(
                    wt[:, :nv],
                    fc2t_d[kc * 128 : (kc + 1) * 128, vc * 512 : vc * 512 + nv],
                )
                fc2_wt[(vc, kc)] = wt

        # ---------------- Wh^T [u, b_loc] ----------------
        # Wh = h_c @ Wattn.T as [16, 1024], then PE-transpose to [u, 16].
        wh_sb = sb_act.tile([BC, U], f32)
        for half in range(2):
            ps = ps_big.tile([128, 512], f32, tag="big", name="ps_wh")[:BC, :]
            for kc in range(UC):
                nc.tensor.matmul(
                    ps[:],
                    hTbc[:, kc * BC : (kc + 1) * BC],
                    waT[kc][:, half * 512 : (half + 1) * 512],
                    start=(kc == 0),
                    stop=(kc == UC - 1),
                )
            nc.scalar.activation(
                wh_sb[:, half * 512 : (half + 1) * 512], ps[:], AF.Copy
            )
        whT = sb_act.tile([128, UC * BC], f32)  # [128, 8*16] col-chunked by uc
        for kc in range(UC):
            pst = ps_sm.tile([128, 128], f32, tag="tp", name="ps_whT")[:, :BC]
            nc.tensor.matmul(
                pst[:],
                wh_sb[:, kc * 128 : (kc + 1) * 128],
                ident32[:BC, :BC],
                is_transpose=True,
            )
            nc.vector.tensor_copy(whT[:, kc * BC : (kc + 1) * BC], pst[:])

        # ---------------- Uf + tanh + score ----------------
        # psum_uf[u_chunk, 512 rows] = featT.T @ uaT chunks; add Wh^T broadcast
        # over l; tanh via ACT (bias = Uattn_b+Wattn_b per-partition);
        # score accumulates v^T @ tanhz over u chunks.
        ps_score = []
        for rg in range(2):
            ps_s = ps_sc.tile([1, 512], f32, tag="sc")
            ps_score.append(ps_s)
            for uc in range(UC):
                ps_uf = ps_big.tile([128, 512], f32, tag="big")
                for ec in range(EC):
                    nc.tensor.matmul(
                        ps_uf[:],
                        uaT[ec][:, uc * 128 : (uc + 1) * 128],
                        featT[ec][:, rg * 512 : (rg + 1) * 512],
                        start=(ec == 0),
                        stop=(ec == EC - 1),
                    )
                # add Wh^T: row rg*512+i has b = (rg*512+i)//64
                wh_bcast = (
                    whT[:, uc * BC + rg * 8 : uc * BC + rg * 8 + 8]
                    .unsqueeze(2)
                    .to_broadcast([128, 8, 64])
                )
                nc.vector.tensor_tensor(
                    out=ps_uf[:].rearrange("p (b l) -> p b l", l=64),
                    in0=ps_uf[:].rearrange("p (b l) -> p b l", l=64),
                    in1=wh_bcast,
                    op=OP.add,
                )
                tz = sb_tz.tile([128, 512], bf16, tag="tanhz")
                nc.scalar.activation(
                    tz[:], ps_uf[:], AF.Tanh, bias=sbias[:, uc : uc + 1]
                )
                nc.tensor.matmul(
                    ps_s[:],
                    vT16[:, uc : uc + 1],
                    tz[:],
                    start=(uc == 0),
                    stop=(uc == UC - 1),
                )

        # ---------------- softmax over l (per b) ----------------
        score_sb = sb_act.tile([1, ROWS], f32)
        for rg in range(2):
            nc.scalar.activation(
                score_sb[:, rg * 512 : (rg + 1) * 512], ps_score[rg][:], AF.Copy
            )
        sc3 = score_sb[:].rearrange("p (b l) -> p b l", l=64)
        mx = sb_act.tile([1, BC], f32)
        nc.vector.reduce_max(mx[:], sc3, axis=mybir.AxisListType.X)
        mx_b = mx[:].unsqueeze(2).to_broadcast([1, BC, 64])
        nc.vector.tensor_tensor(out=sc3, in0=sc3, in1=mx_b, op=OP.subtract)
        esb = sb_act.tile([1, ROWS], f32)
        nc.scalar.activation(esb[:], score_sb[:], AF.Exp)
        ssum = sb_act.tile([1, BC], f32)
        nc.vector.reduce_sum(
            ssum[:], esb[:].rearrange("p (b l) -> p b l", l=64), axis=mybir.AxisListType.X
        )
        rsum = sb_act.tile([1, BC], f32)
        nc.vector.reciprocal(rsum[:], ssum[:])
        attn_sb = sb_act.tile([1, ROWS], f32)
        nc.vector.tensor_tensor(
            out=attn_sb[:].rearrange("p (b l) -> p b l", l=64),
            in0=esb[:].rearrange("p (b l) -> p b l", l=64),
            in1=rsum[:].unsqueeze(2).to_broadcast([1, BC, 64]),
            op=OP.mult,
        )
        nc.scalar.dma_start(attn_d[:, :], attn_sb[:])
        attn16 = sb_act.tile([1, ROWS], bf16)
        nc.vector.tensor_copy(attn16[:], attn_sb[:])

        # ---------------- ctx = attn-weighted sum of features ----------------
        # Build block "diagonal" lhsT tiles: Ablk[:, rt*16+b] holds attn[b, :]
        # at partitions (b-2rt)*64..+64 for b in {2rt, 2rt+1}, zeros elsewhere.
        ablk = sb_act.tile([128, RT * BC], bf16)
        nc.vector.memset(ablk[:], 0.0)
        for rt in range(RT):
            pst = ps_sm.tile([128, 256], bf16, tag="tp", name="ps_attnT")[:, :1]
            nc.tensor.matmul(
                pst[:],
                attn16[:, rt * 128 : (rt + 1) * 128],
                ident16[:1, :1],
                is_transpose=True,
            )
            b0 = 2 * rt
            nc.vector.tensor_copy(
                ablk[0:64, rt * BC + b0 : rt * BC + b0 + 1], pst[0:64, :]
            )
            nc.vector.tensor_copy(
                ablk[64:128, rt * BC + b0 + 1 : rt * BC + b0 + 2], pst[64:128, :]
            )
        ps_ctx = ps_big.tile([128, 512], f32, tag="big", name="ps_ctx")[:BC, :EMB]
        for rt in range(RT):
            nc.tensor.matmul(
                ps_ctx[:],
                ablk[:, rt * BC : (rt + 1) * BC],
                featN[rt][:],
                start=(rt == 0),
                stop=(rt == RT - 1),
            )
        ctx_sb = sb_act.tile([BC, EMB], bf16)
        nc.scalar.activation(ctx_sb[:], ps_ctx[:], AF.Copy)

        # ---------------- AllGather ctx -> full batch (bf16) ----------------
        ctx_bounce = dram.tile([BC, EMB], bf16)
        ctx_ag = dram.tile([B, EMB], bf16)
        nc.scalar.dma_start(ctx_bounce[:], ctx_sb[:])
        nc.gpsimd.collective_compute(
            "AllGather",
            OP.bypass,
            replica_groups=groups,
            ins=[ctx_bounce.opt()],
            outs=[ctx_ag.opt()],
        )

        # ---------------- ginT (bf16): ctx^T chunks + e^T chunks ----------------
        ginT = sb_act.tile([128, UC * B], bf16)  # col-chunked [kc][128, 128]
        cg_sb = sb_act.tile([128, EMB], bf16)
        nc.scalar.dma_start(cg_sb[:], ctx_ag[:, :])
        for ec in range(EC):
            pst = ps_sm.tile([128, 128], bf16, tag="tp", name="pst_gin")
            nc.tensor.matmul(
                pst[:],
                cg_sb[:, ec * 128 : (ec + 1) * 128],
                ident16[:, :],
                is_transpose=True,
            )
            nc.vector.tensor_copy(ginT[:, ec * B : (ec + 1) * B], pst[:])
        nc.sync.dma_start(
            ginT[:, EC * B :].rearrange("p (kc b) -> p kc b", kc=EC),
            eT16_d.rearrange("(kc p) b -> p kc b", p=128),
        )
        # ---------------- GRU gates (TP slice of 128 u per core) ----------------
        ps_rz = ps_big.tile([128, 512], f32, tag="big", name="ps_rz")[:, : 2 * US]
        n_mm = 2 * UC + 2
        i_mm = 0
        for kc in range(UC):
            nc.tensor.matmul(
                ps_rz[:],
                hT_full[kc][:],
                wrzhh[:, kc * 2 * US : (kc + 1) * 2 * US],
                start=(i_mm == 0),
                stop=(i_mm == n_mm - 1),
            )
            i_mm += 1
        for kc in [4, 5, 6, 7, 0, 1, 2, 3]:
            nc.tensor.matmul(
                ps_rz[:],
                ginT[:, kc * B : (kc + 1) * B],
                wrzih[:, kc * 2 * US : (kc + 1) * 2 * US],
                start=(i_mm == 0),
                stop=(i_mm == n_mm - 1),
            )
            i_mm += 1
        nc.tensor.matmul(ps_rz[:], ones16[:], brz_hi[:], start=False, stop=False)
        nc.tensor.matmul(ps_rz[:], ones16[:], brz_lo[:], start=False, stop=True)

        ps_hn = ps_sm.tile([128, 128], f32, tag="tp")
        for kc in range(UC):
            nc.tensor.matmul(
                ps_hn[:],
                hT_full[kc][:],
                wnhh[:, kc * US : (kc + 1) * US],
                start=(kc == 0),
                stop=False,
            )
        nc.tensor.matmul(ps_hn[:], ones16[:], bhn_hi[:], start=False, stop=False)
        nc.tensor.matmul(ps_hn[:], ones16[:], bhn_lo[:], start=False, stop=True)

        ps_in = ps_sm.tile([128, 128], f32, tag="tp")
        for j, kc in enumerate([4, 5, 6, 7, 0, 1, 2, 3]):
            nc.tensor.matmul(
                ps_in[:],
                ginT[:, kc * B : (kc + 1) * B],
                wnih[:, kc * US : (kc + 1) * US],
                start=(j == 0),
                stop=False,
            )
        nc.tensor.matmul(ps_in[:], ones16[:], bin_hi[:], start=False, stop=False)
        nc.tensor.matmul(ps_in[:], ones16[:], bin_lo[:], start=False, stop=True)

        r_sb = sb_act.tile([B, US], f32)
        nc.scalar.activation(r_sb[:], ps_rz[:, 0:US], AF.Sigmoid)
        z_sb = sb_act.tile([B, US], f32)
        nc.scalar.activation(z_sb[:], ps_rz[:, US : 2 * US], AF.Sigmoid)
        rhn = sb_act.tile([B, US], f32)
        nc.vector.tensor_tensor(out=rhn[:], in0=ps_hn[:], in1=r_sb[:], op=OP.mult)
        nc.vector.tensor_tensor(out=ps_in[:], in0=ps_in[:], in1=rhn[:], op=OP.add)
        n_sb = sb_act.tile([B, US], f32)
        nc.scalar.activation(n_sb[:], ps_in[:], AF.Tanh)
        hU = sb_act.tile([B, US], f32)
        nc.scalar.dma_start(hU[:], hU_d[:, :])
        d_sb = sb_act.tile([B, US], f32)
        nc.vector.tensor_tensor(out=d_sb[:], in0=hU[:], in1=n_sb[:], op=OP.subtract)
        zd_sb = sb_act.tile([B, US], f32)
        nc.vector.tensor_tensor(out=zd_sb[:], in0=z_sb[:], in1=d_sb[:], op=OP.mult)
        hnew_sb = sb_act.tile([B, US], f32)
        nc.vector.tensor_tensor(out=hnew_sb[:], in0=n_sb[:], in1=zd_sb[:], op=OP.add)
        nc.scalar.dma_start(hnew_d[:, :], hnew_sb[:])

        # ---------------- AllGather h_new ----------------
        hnew_bounce = dram.tile([B, US], f32)
        hnew_ag = dram.tile([N_CORES * B, US], f32)  # blocks [c][b, u_loc]
        nc.scalar.dma_start(hnew_bounce[:], hnew_sb[:])
        nc.gpsimd.collective_compute(
            "AllGather",
            OP.bypass,
            replica_groups=groups,
            ins=[hnew_bounce.opt()],
            outs=[hnew_ag.opt()],
        )

        # transpose blocks to h_new^T [u, b] chunks (f32 for the fc matmul)
        hnT = sb_act.tile([128, UC * B], bf16)
        for c2 in range(N_CORES):
            blk = sb_tz.tile([128, US], f32, tag="hn_blk")
            nc.scalar.dma_start(blk[:], hnew_ag[c2 * B : (c2 + 1) * B, :])
            pst = ps_sm.tile([128, 128], f32, tag="tp")
            nc.tensor.matmul(pst[:], blk[:], ident32[:, :], is_transpose=True)
            nc.vector.tensor_copy(hnT[:, c2 * B : (c2 + 1) * B], pst[:])

        # ---------------- y = h_new @ fc_w^T (BN folded), replicated ----------------
        ybn_sb = sb_act.tile([B, U], f32)
        for half in range(2):
            ps_y = ps_big.tile([128, 512], f32, tag="big", name="ps_y")
            for kc in range(UC):
                nc.tensor.matmul(
                    ps_y[:],
                    hnT[:, kc * B : (kc + 1) * B],
                    fcw[:, kc * U + half * 512 : kc * U + (half + 1) * 512],
                    start=(kc == 0),
                    stop=(kc == UC - 1),
                )
            nc.vector.tensor_copy(ybn_sb[:, half * 512 : (half + 1) * 512], ps_y[:])

        fc2_in_dt = bf16 if FC2_BF16 else f32
        ybnT = sb_act.tile([128, UC * B], fc2_in_dt)
        for c2 in range(N_CORES):
            pst = ps_sm.tile([128, 128], f32, tag="tp")
            nc.tensor.matmul(
                pst[:], ybn_sb[:, c2 * US : (c2 + 1) * US], ident32[:, :],
                is_transpose=True,
            )
            nc.vector.tensor_copy(ybnT[:, c2 * B : (c2 + 1) * B], pst[:])

        # ---------------- fc2: logits = ybn @ fc2_w^T + fc2_b ----------------
        for vc in range(NVC):
            nv = min(512, VS - vc * 512)
            ps_l = ps_big.tile([128, 512], f32, tag="big")
            for kc in range(UC):
                wt = fc2_wt[(vc, kc)]
                nc.tensor.matmul(
                    ps_l[:, :nv],
                    ybnT[:, kc * B : (kc + 1) * B],
                    wt[:, :nv],
                    start=(kc == 0),
                    stop=False,
                )
            if FC2_BF16:
                nc.tensor.matmul(
                    ps_l[:, :nv],
                    ones16[:],
                    fc2bh[:, vc * 512 : vc * 512 + nv],
                    start=False,
                    stop=False,
                )
                nc.tensor.matmul(
                    ps_l[:, :nv],
                    ones16[:],
                    fc2bl[:, vc * 512 : vc * 512 + nv],
                    start=False,
                    stop=True,
                )
            else:
                nc.tensor.matmul(
                    ps_l[:, :nv],
                    ones32[:],
                    fc2b[:, vc * 512 : vc * 512 + nv],
                    start=False,
                    stop=True,
                )
            lg = sb_out.tile([B, 512], f32, tag="lg")
            nc.vector.tensor_copy(lg[:, :nv], ps_l[:, :nv])
            nc.scalar.dma_start(logits_d[:, vc * 512 : vc * 512 + nv], lg[:, :nv])

        ctxmgr.close()

    nc.compile()
    return nc


def _host_prep(inputs):
    import ml_dtypes

    bf16 = ml_dtypes.bfloat16
    f32 = np.float32

    x = np.asarray(inputs["x"])
    features = np.asarray(inputs["features"], dtype=f32)
    hidden = np.asarray(inputs["hidden"], dtype=f32)
    emb_table = np.asarray(inputs["emb_table"], dtype=f32)
    Uattn_w = np.asarray(inputs["Uattn_w"], dtype=f32)
    Uattn_b = np.asarray(inputs["Uattn_b"], dtype=f32)
    Wattn_w = np.asarray(inputs["Wattn_w"], dtype=f32)
    Wattn_b = np.asarray(inputs["Wattn_b"], dtype=f32)
    Vattn_w = np.asarray(inputs["Vattn_w"], dtype=f32)
    W_ih = np.asarray(inputs["W_ih"], dtype=f32)
    W_hh = np.asarray(inputs["W_hh"], dtype=f32)
    b_ih = np.asarray(inputs["b_ih"], dtype=f32)
    b_hh = np.asarray(inputs["b_hh"], dtype=f32)
    fc_w = np.asarray(inputs["fc_w"], dtype=f32)
    fc_b = np.asarray(inputs["fc_b"], dtype=f32)
    bn_gamma = np.asarray(inputs["bn_gamma"], dtype=f32)
    bn_beta = np.asarray(inputs["bn_beta"], dtype=f32)
    bn_mean = np.asarray(inputs["bn_mean"], dtype=f32)
    bn_var = np.asarray(inputs["bn_var"], dtype=f32)
    fc2_w = np.asarray(inputs["fc2_w"], dtype=f32)
    fc2_b = np.asarray(inputs["fc2_b"], dtype=f32)

    h = hidden[0]  # [B, U]
    hT16 = np.ascontiguousarray(h.T).astype(bf16)  # [U, B]
    uaT16 = np.ascontiguousarray(Uattn_w.T).astype(bf16)  # [EMB, U]
    waT16 = np.ascontiguousarray(Wattn_w.T).astype(bf16)  # [U, U]
    sbias = np.ascontiguousarray((Uattn_b + Wattn_b).reshape(UC, 128).T)  # [128, UC]
    vT16 = np.ascontiguousarray(Vattn_w[0].reshape(UC, 128).T).astype(bf16)
    e_full = emb_table[x[:, 0].astype(np.int64)]  # [B, EMB] row gather
    eT16 = np.ascontiguousarray(e_full.T).astype(bf16)  # [EMB, B]

    W_ihT = np.ascontiguousarray(W_ih.T)  # [U, 3U]
    W_hhT = np.ascontiguousarray(W_hh.T)
    bsum = (b_ih.astype(np.float64) + b_hh.astype(np.float64))  # [3U]

    bn_s = bn_gamma / np.sqrt(bn_var + BN_EPS)
    bn_t = bn_beta - bn_mean * bn_s
    fc_wTs16 = np.ascontiguousarray(fc_w.T * bn_s[None, :]).astype(bf16)  # [U, U]
    t2_full = (
        fc_b.astype(np.float64) * bn_s.astype(np.float64) + bn_t.astype(np.float64)
    )  # [U]

    fc2T = np.zeros((U, N_CORES * VS), dtype=f32)
    fc2T[:, :V] = fc2_w.T
    fc2b_pad = np.zeros(N_CORES * VS, dtype=np.float64)
    fc2b_pad[:V] = fc2_b.astype(np.float64) + t2_full @ fc2_w.T.astype(np.float64)

    def hilo(v):  # bf16 hi/lo split of a float64 vector -> [1, n] each
        hi = v.astype(bf16)
        lo = (v - hi.astype(np.float64)).astype(bf16)
        return hi.reshape(1, -1), lo.reshape(1, -1)

    in_maps = []
    for c in range(N_CORES):
        bc = slice(c * BC, (c + 1) * BC)
        Sc = slice(c * US, (c + 1) * US)
        feat_c = np.ascontiguousarray(features[bc].reshape(ROWS, EMB))
        m = {
            "feat16": feat_c.astype(bf16),
            "featT16": np.ascontiguousarray(feat_c.T).astype(bf16),
            "uaT16": uaT16,
            "waT16": waT16,
            "hT16": hT16,
            "hTbc16": np.ascontiguousarray(hT16[:, bc]),
            "hU": np.ascontiguousarray(h[:, Sc]),
            "sbias": sbias,
            "vT16": vT16,
            "eT16": eT16,
            "wrzih": np.ascontiguousarray(
                np.hstack([W_ihT[:, Sc], W_ihT[:, U + c * US : U + (c + 1) * US]])
            ).astype(bf16),
            "wrzhh": np.ascontiguousarray(
                np.hstack([W_hhT[:, Sc], W_hhT[:, U + c * US : U + (c + 1) * US]])
            ).astype(bf16),
            "wnih": np.ascontiguousarray(
                W_ihT[:, 2 * U + c * US : 2 * U + (c + 1) * US]
            ).astype(bf16),
            "wnhh": np.ascontiguousarray(
                W_hhT[:, 2 * U + c * US : 2 * U + (c + 1) * US]
            ).astype(bf16),
            "fcw": fc_wTs16,
            "fc2t": np.ascontiguousarray(fc2T[:, c * VS : (c + 1) * VS]).astype(
                bf16 if FC2_BF16 else f32
            ),
        }
        brz = np.concatenate([bsum[Sc], bsum[U + c * US : U + (c + 1) * US]])
        m["brz_hi"], m["brz_lo"] = hilo(brz)
        m["bin_hi"], m["bin_lo"] = hilo(
            b_ih[2 * U + c * US : 2 * U + (c + 1) * US].astype(np.float64)
        )
        m["bhn_hi"], m["bhn_lo"] = hilo(
            b_hh[2 * U + c * US : 2 * U + (c + 1) * US].astype(np.float64)
        )
        fb = fc2b_pad[c * VS : (c + 1) * VS]
        if FC2_BF16:
            m["fc2bh"], m["fc2bl"] = hilo(fb)
        else:
            m["fc2b"] = fb.astype(f32).reshape(1, VS)
        in_maps.append(m)
    return in_maps


def kernel(**inputs):
    from concourse.bass_utils import run_bass_kernel_spmd

    if "nc" not in _CACHE:
        _CACHE["nc"] = _build_bass()
    nc = _CACHE["nc"]
    in_maps = _host_prep(inputs)
    res = run_bass_kernel_spmd(nc, in_maps, list(range(N_CORES)))
    rs = res.results
    logits = np.concatenate([rs[c]["logits_c"] for c in range(N_CORES)], axis=1)[
        :, :V
    ]
    h_new = np.concatenate([rs[c]["hnew_c"] for c in range(N_CORES)], axis=1)[
        None, :, :
    ]
    attn = np.concatenate([rs[c]["attn_c"] for c in range(N_CORES)], axis=0)[
        :, :, None
    ]
    return logits.astype(np.float32), h_new.astype(np.float32), attn.astype(
        np.float32
    )


# revision 13
# speedup vs baseline: 1.7103x; 1.0572x over previous
"""TRN2 Bass kernel for nn_Decode (Bahdanau-attention GRU decode step + vocab head).

Sharding across 8 NeuronCores (SPMD, one program, per-core data):
  - Attention: data-parallel over batch (16 rows/core); small attention weights
    replicated (shipped bf16).
  - GRU + fc/BN: tensor-parallel over the hidden dim (128 of 1024 per core),
    stitched with AllGather collectives.
  - fc2 (vocab head) + embedding gather: tensor-parallel over vocab
    (6283 of 50257 rows per core).
  - Host does layout prep only: weight transposes/casts, embedding row gather,
    BN constant folding, output concatenation.

Numerics: bf16 operands for attention/GRU matmuls (≈1e-4 relative error on
outputs), fp32 for the fc/BN/fc2 chain unless FC2_BF16 is set.
"""

import numpy as np

N_CORES = 8
B, L, EMB, U, V = 128, 64, 512, 1024, 50257
BC = B // N_CORES  # 16 batch rows per core
US = U // N_CORES  # 128 hidden slice per core (GRU/fc TP)
VS = -(-V // N_CORES)  # 6283 vocab rows per core
BN_EPS = 1e-3

FC2_BF16 = True  # fc2 weights/matmul in bf16 (halves the dominant DMA)

ROWS = BC * L  # 1024 attention rows per core
RT = ROWS // 128  # 8 row chunks
EC = EMB // 128  # 4 embedding chunks
UC = U // 128  # 8 hidden chunks
NVC = -(-VS // 512)  # 13 vocab column chunks (last is 139 wide)

_CACHE = {}


def _build_bass():
    import concourse.tile as tile
    from concourse import bacc, masks, mybir

    f32 = mybir.dt.float32
    bf16 = mybir.dt.bfloat16
    AF = mybir.ActivationFunctionType
    OP = mybir.AluOpType
    fc2_dt = bf16 if FC2_BF16 else f32

    nc = bacc.Bacc(None, target_bir_lowering=False, num_devices=N_CORES)

    din = lambda n, s, d: nc.dram_tensor(n, s, d, kind="ExternalInput")
    feat16_d = din("feat16", [ROWS, EMB], bf16)  # this core's features, natural
    featT16_d = din("featT16", [EMB, ROWS], bf16)  # transposed
    uaT16_d = din("uaT16", [EMB, U], bf16)  # Uattn_w.T (replicated)
    waT16_d = din("waT16", [U, U], bf16)  # Wattn_w.T (replicated)
    hT16_d = din("hT16", [U, B], bf16)  # full h.T (replicated)
    hTbc16_d = din("hTbc16", [U, BC], bf16)  # this core's batch cols of h.T
    hU_d = din("hU", [B, US], f32)  # this core's hidden-slice of h, natural
    sbias_d = din("sbias", [128, UC], f32)  # (Uattn_b+Wattn_b) col-chunked
    vT16_d = din("vT16", [128, UC], bf16)  # Vattn_w col-chunked
    eT16_d = din("eT16", [EMB, B], bf16)  # gathered embeddings.T (replicated)
    wrzih_d = din("wrzih", [U, 2 * US], bf16)  # W_ih.T r|z slices
    wrzhh_d = din("wrzhh", [U, 2 * US], bf16)
    wnih_d = din("wnih", [U, US], bf16)  # W_ih.T n slice
    wnhh_d = din("wnhh", [U, US], bf16)
    brz_hi_d = din("brz_hi", [1, 2 * US], bf16)  # (b_ih+b_hh) r|z hi/lo
    brz_lo_d = din("brz_lo", [1, 2 * US], bf16)
    bin_hi_d = din("bin_hi", [1, US], bf16)  # b_ih n slice hi/lo
    bin_lo_d = din("bin_lo", [1, US], bf16)
    bhn_hi_d = din("bhn_hi", [1, US], bf16)  # b_hh n slice hi/lo
    bhn_lo_d = din("bhn_lo", [1, US], bf16)
    fcw_d = din("fcw", [U, U], bf16)  # (fc_w.T * bn_scale), full, replicated
    fc2t_d = din("fc2t", [U, VS], fc2_dt)  # fc2_w.T vocab slice
    if FC2_BF16:
        fc2bh_d = din("fc2bh", [1, VS], bf16)
        fc2bl_d = din("fc2bl", [1, VS], bf16)
    else:
        fc2b_d = din("fc2b", [1, VS], f32)

    logits_d = nc.dram_tensor("logits_c", [B, VS], f32, kind="ExternalOutput")
    hnew_d = nc.dram_tensor("hnew_c", [B, US], f32, kind="ExternalOutput")
    attn_d = nc.dram_tensor("attn_c", [BC, L], f32, kind="ExternalOutput")

    groups = [list(range(N_CORES))]

    with tile.TileContext(nc) as tc:
        from contextlib import ExitStack

        ctxmgr = ExitStack()
        consts = ctxmgr.enter_context(tc.tile_pool(name="consts", bufs=1))
        sb_w = ctxmgr.enter_context(tc.tile_pool(name="weights", bufs=1))
        sb_act = ctxmgr.enter_context(tc.tile_pool(name="acts", bufs=1))
        sb_tz = ctxmgr.enter_context(tc.tile_pool(name="tanhz", bufs=3))
        sb_out = ctxmgr.enter_context(tc.tile_pool(name="outs", bufs=3))
        ps_big = ctxmgr.enter_context(tc.tile_pool(name="ps_big", bufs=2, space="PSUM"))
        ps_sc = ctxmgr.enter_context(tc.tile_pool(name="ps_sc", bufs=2, space="PSUM"))
        ps_sm = ps_big
        dram = ctxmgr.enter_context(tc.tile_pool(name="dram", bufs=1, space="DRAM"))
        fc2_pool = ctxmgr.enter_context(tc.tile_pool(name="fc2s", bufs=72))

        ident32 = consts.tile([128, 128], f32)
        masks.make_identity(nc, ident32[:])
        ident16 = consts.tile([128, 128], bf16)
        masks.make_identity(nc, ident16[:])
        ones16 = consts.tile([1, 128], bf16)
        nc.vector.memset(ones16[:], 1.0)
        ones32 = consts.tile([1, 128], f32)
        nc.vector.memset(ones32[:], 1.0)

        # ---------------- core-alignment dummy collective ----------------
        align_in = dram.tile([1, 8], f32)
        align_out = dram.tile([N_CORES, 8], f32)
        nc.gpsimd.dma_start(align_in[:], ones32[:, 0:8])
        nc.gpsimd.collective_compute(
            "AllGather",
            OP.bypass,
            replica_groups=groups,
            ins=[align_in.opt()],
            outs=[align_out.opt()],
        )

        # ---------------- load attention inputs ----------------
        featT = []
        for ec in range(EC):
            t = sb_act.tile([128, ROWS], bf16, tag=f"featT{ec}")
            nc.sync.dma_start(t[:], featT16_d[ec * 128 : (ec + 1) * 128, :])
            featT.append(t)
        uaT = []
        for ec in range(EC):
            t = sb_w.tile([128, U], bf16, tag=f"uaT{ec}")
            nc.sync.dma_start(t[:], uaT16_d[ec * 128 : (ec + 1) * 128, :])
            uaT.append(t)
        featN = []
        for rt in range(RT):
            t = sb_act.tile([128, EMB], bf16, tag=f"featN{rt}")
            nc.sync.dma_start(t[:], feat16_d[rt * 128 : (rt + 1) * 128, :])
            featN.append(t)
        waT = []
        for kc in range(UC):
            t = sb_w.tile([128, U], bf16, tag=f"waT{kc}")
            nc.sync.dma_start(t[:], waT16_d[kc * 128 : (kc + 1) * 128, :])
            waT.append(t)
        hTbc = sb_act.tile([128, UC * BC], bf16)  # [128, 8*16] col-chunked
        nc.sync.dma_start(
            hTbc[:].rearrange("p (kc b) -> p kc b", kc=UC),
            hTbc16_d.rearrange("(kc p) b -> p kc b", p=128),
        )
        sbias = consts.tile([128, UC], f32)
        nc.sync.dma_start(sbias[:], sbias_d[:, :])
        vT16 = consts.tile([128, UC], bf16)
        nc.sync.dma_start(vT16[:], vT16_d[:, :])

        # ---------------- early streaming loads (pure, dep-free) ----------------
        hT_full = []  # bf16 [128, 128] chunks of h^T for the gh matmuls
        for kc in range(UC):
            t = sb_act.tile([128, B], bf16, tag=f"hTf{kc}")
            nc.sync.dma_start(t[:], hT16_d[kc * 128 : (kc + 1) * 128, :])
            hT_full.append(t)

        wrzih = sb_w.tile([128, UC * 2 * US], bf16)
        nc.sync.dma_start(
            wrzih[:].rearrange("p (kc n) -> p kc n", kc=UC),
            wrzih_d.rearrange("(kc p) n -> p kc n", p=128),
        )
        wrzhh = sb_w.tile([128, UC * 2 * US], bf16)
        nc.sync.dma_start(
            wrzhh[:].rearrange("p (kc n) -> p kc n", kc=UC),
            wrzhh_d.rearrange("(kc p) n -> p kc n", p=128),
        )
        wnih = sb_w.tile([128, UC * US], bf16)
        nc.sync.dma_start(
            wnih[:].rearrange("p (kc n) -> p kc n", kc=UC),
            wnih_d.rearrange("(kc p) n -> p kc n", p=128),
        )
        wnhh = sb_w.tile([128, UC * US], bf16)
        nc.sync.dma_start(
            wnhh[:].rearrange("p (kc n) -> p kc n", kc=UC),
            wnhh_d.rearrange("(kc p) n -> p kc n", p=128),
        )
        brz_hi = sb_w.tile([1, 2 * US], bf16)
        nc.sync.dma_start(brz_hi[:], brz_hi_d[:, :])
        brz_lo = sb_w.tile([1, 2 * US], bf16)
        nc.sync.dma_start(brz_lo[:], brz_lo_d[:, :])
        bin_hi = sb_w.tile([1, US], bf16)
        nc.sync.dma_start(bin_hi[:], bin_hi_d[:, :])
        bin_lo = sb_w.tile([1, US], bf16)
        nc.sync.dma_start(bin_lo[:], bin_lo_d[:, :])
        bhn_hi = sb_w.tile([1, US], bf16)
        nc.sync.dma_start(bhn_hi[:], bhn_hi_d[:, :])
        bhn_lo = sb_w.tile([1, US], bf16)
        nc.sync.dma_start(bhn_lo[:], bhn_lo_d[:, :])

        if FC2_BF16:
            fc2bh = sb_w.tile([1, VS], bf16)
            nc.sync.dma_start(fc2bh[:], fc2bh_d[:, :])
            fc2bl = sb_w.tile([1, VS], bf16)
            nc.sync.dma_start(fc2bl[:], fc2bl_d[:, :])
        else:
            fc2b = sb_w.tile([1, VS], f32)
            nc.sync.dma_start(fc2b[:], fc2b_d[:, :])

        # fc weight stream tiles (consumed first), then fc2 stream
        fcw_wt = {}
        for half in range(2):
            for kc in range(UC):
                wt = fc2_pool.tile([128, 512], bf16, tag="fc2w", name=f"fcw{half}_{kc}")
                nc.sync.dma_start(
                    wt[:],
                    fcw_d[kc * 128 : (kc + 1) * 128, half * 512 : (half + 1) * 512],
                )
                fcw_wt[(half, kc)] = wt
        # fc2 weight stream: prefetched tiles, consumed in order by the matmuls
        fc2_wt = {}
        for vc in range(NVC):
            nv = min(512, VS - vc * 512)
            for kc in range(UC):
                wt = fc2_pool.tile([128, 512], fc2_dt, tag="fc2w", name=f"wt{vc}_{kc}")
                nc.sync.dma_start(
                    wt[:, :nv],
                    fc2t_d[kc * 128 : (kc + 1) * 128, vc * 512 : vc * 512 + nv],
                )
                fc2_wt[(vc, kc)] = wt

        # ---------------- Wh^T [u, b_loc] ----------------
        # Wh = h_c @ Wattn.T as [16, 1024], then PE-transpose to [u, 16].
        wh_sb = sb_act.tile([BC, U], f32)
        for half in range(2):
            ps = ps_big.tile([128, 512], f32, tag="big", name="ps_wh")[:BC, :]
            for kc in range(UC):
                nc.tensor.matmul(
                    ps[:],
                    hTbc[:, kc * BC : (kc + 1) * BC],
                    waT[kc][:, half * 512 : (half + 1) * 512],
                    start=(kc == 0),
                    stop=(kc == UC - 1),
                )
            nc.scalar.activation(
                wh_sb[:, half * 512 : (half + 1) * 512], ps[:], AF.Copy
            )
        whT = sb_act.tile([128, UC * BC], f32)  # [128, 8*16] col-chunked by uc
        for kc in range(UC):
            pst = ps_sm.tile([128, 128], f32, tag="tp", name="ps_whT")[:, :BC]
            nc.tensor.matmul(
                pst[:],
                wh_sb[:, kc * 128 : (kc + 1) * 128],
                ident32[:BC, :BC],
                is_transpose=True,
            )
            nc.vector.tensor_copy(whT[:, kc * BC : (kc + 1) * BC], pst[:])

        # ---------------- Uf + tanh + score ----------------
        # psum_uf[u_chunk, 512 rows] = featT.T @ uaT chunks; add Wh^T broadcast
        # over l; tanh via ACT (bias = Uattn_b+Wattn_b per-partition);
        # score accumulates v^T @ tanhz over u chunks.
        ps_score = []
        for rg in range(2):
            ps_s = ps_sc.tile([1, 512], f32, tag="sc")
            ps_score.append(ps_s)
            for uc in range(UC):
                ps_uf = ps_big.tile([128, 512], f32, tag="big")
                for ec in range(EC):
                    nc.tensor.matmul(
                        ps_uf[:],
                        uaT[ec][:, uc * 128 : (uc + 1) * 128],
                        featT[ec][:, rg * 512 : (rg + 1) * 512],
                        start=(ec == 0),
                        stop=(ec == EC - 1),
                    )
                # add Wh^T: row rg*512+i has b = (rg*512+i)//64
                wh_bcast = (
                    whT[:, uc * BC + rg * 8 : uc * BC + rg * 8 + 8]
                    .unsqueeze(2)
                    .to_broadcast([128, 8, 64])
                )
                nc.vector.tensor_tensor(
                    out=ps_uf[:].rearrange("p (b l) -> p b l", l=64),
                    in0=ps_uf[:].rearrange("p (b l) -> p b l", l=64),
                    in1=wh_bcast,
                    op=OP.add,
                )
                tz = sb_tz.tile([128, 512], bf16, tag="tanhz")
                nc.scalar.activation(
                    tz[:], ps_uf[:], AF.Tanh, bias=sbias[:, uc : uc + 1]
                )
                nc.tensor.matmul(
                    ps_s[:],
                    vT16[:, uc : uc + 1],
                    tz[:],
                    start=(uc == 0),
                    stop=(uc == UC - 1),
                )

        # ---------------- softmax over l (per b) ----------------
        score_sb = sb_act.tile([1, ROWS], f32)
        for rg in range(2):
            nc.scalar.activation(
                score_sb[:, rg * 512 : (rg + 1) * 512], ps_score[rg][:], AF.Copy
            )
        sc3 = score_sb[:].rearrange("p (b l) -> p b l", l=64)
        mx = sb_act.tile([1, BC], f32)
        nc.vector.reduce_max(mx[:], sc3, axis=mybir.AxisListType.X)
        mx_b = mx[:].unsqueeze(2).to_broadcast([1, BC, 64])
        nc.vector.tensor_tensor(out=sc3, in0=sc3, in1=mx_b, op=OP.subtract)
        esb = sb_act.tile([1, ROWS], f32)
        nc.scalar.activation(esb[:], score_sb[:], AF.Exp)
        ssum = sb_act.tile([1, BC], f32)
        nc.vector.reduce_sum(
            ssum[:], esb[:].rearrange("p (b l) -> p b l", l=64), axis=mybir.AxisListType.X
        )
        rsum = sb_act.tile([1, BC], f32)
        nc.vector.reciprocal(rsum[:], ssum[:])
        attn_sb = sb_act.tile([1, ROWS], f32)
        nc.vector.tensor_tensor(
            out=attn_sb[:].rearrange("p (b l) -> p b l", l=64),
            in0=esb[:].rearrange("p (b l) -> p b l", l=64),
            in1=rsum[:].unsqueeze(2).to_broadcast([1, BC, 64]),
            op=OP.mult,
        )
        nc.scalar.dma_start(attn_d[:, :], attn_sb[:])
        attn16 = sb_act.tile([1, ROWS], bf16)
        nc.vector.tensor_copy(attn16[:], attn_sb[:])

        # ---------------- ctx = attn-weighted sum of features ----------------
        # Build block "diagonal" lhsT tiles: Ablk[:, rt*16+b] holds attn[b, :]
        # at partitions (b-2rt)*64..+64 for b in {2rt, 2rt+1}, zeros elsewhere.
        ablk = sb_act.tile([128, RT * BC], bf16)
        nc.vector.memset(ablk[:], 0.0)
        for rt in range(RT):
            pst = ps_sm.tile([128, 256], bf16, tag="tp", name="ps_attnT")[:, :1]
            nc.tensor.matmul(
                pst[:],
                attn16[:, rt * 128 : (rt + 1) * 128],
                ident16[:1, :1],
                is_transpose=True,
            )
            b0 = 2 * rt
            nc.vector.tensor_copy(
                ablk[0:64, rt * BC + b0 : rt * BC + b0 + 1], pst[0:64, :]
            )
            nc.vector.tensor_copy(
                ablk[64:128, rt * BC + b0 + 1 : rt * BC + b0 + 2], pst[64:128, :]
            )
        ps_ctx = ps_big.tile([128, 512], f32, tag="big", name="ps_ctx")[:BC, :EMB]
        for rt in range(RT):
            nc.tensor.matmul(
                ps_ctx[:],
                ablk[:, rt * BC : (rt + 1) * BC],
                featN[rt][:],
                start=(rt == 0),
                stop=(rt == RT - 1),
            )
        ctx_sb = sb_act.tile([BC, EMB], bf16)
        nc.scalar.activation(ctx_sb[:], ps_ctx[:], AF.Copy)

        # ---------------- AllGather ctx -> full batch (bf16) ----------------
        ctx_bounce = dram.tile([BC, EMB], bf16)
        ctx_ag = dram.tile([B, EMB], bf16)
        nc.scalar.dma_start(ctx_bounce[:], ctx_sb[:])
        nc.gpsimd.collective_compute(
            "AllGather",
            OP.bypass,
            replica_groups=groups,
            ins=[ctx_bounce.opt()],
            outs=[ctx_ag.opt()],
        )

        # ---------------- ginT (bf16): ctx^T chunks + e^T chunks ----------------
        ginT = sb_act.tile([128, UC * B], bf16)  # col-chunked [kc][128, 128]
        cg_sb = sb_act.tile([128, EMB], bf16)
        nc.scalar.dma_start(cg_sb[:], ctx_ag[:, :])
        for ec in range(EC):
            pst = ps_sm.tile([128, 128], bf16, tag="tp", name="pst_gin")
            nc.tensor.matmul(
                pst[:],
                cg_sb[:, ec * 128 : (ec + 1) * 128],
                ident16[:, :],
                is_transpose=True,
            )
            nc.vector.tensor_copy(ginT[:, ec * B : (ec + 1) * B], pst[:])
        nc.sync.dma_start(
            ginT[:, EC * B :].rearrange("p (kc b) -> p kc b", kc=EC),
            eT16_d.rearrange("(kc p) b -> p kc b", p=128),
        )
        # ---------------- GRU gates (TP slice of 128 u per core) ----------------
        ps_rz = ps_big.tile([128, 512], f32, tag="big", name="ps_rz")[:, : 2 * US]
        n_mm = 2 * UC + 2
        i_mm = 0
        for kc in range(UC):
            nc.tensor.matmul(
                ps_rz[:],
                hT_full[kc][:],
                wrzhh[:, kc * 2 * US : (kc + 1) * 2 * US],
                start=(i_mm == 0),
                stop=(i_mm == n_mm - 1),
            )
            i_mm += 1
        for kc in [4, 5, 6, 7, 0, 1, 2, 3]:
            nc.tensor.matmul(
                ps_rz[:],
                ginT[:, kc * B : (kc + 1) * B],
                wrzih[:, kc * 2 * US : (kc + 1) * 2 * US],
                start=(i_mm == 0),
                stop=(i_mm == n_mm - 1),
            )
            i_mm += 1
        nc.tensor.matmul(ps_rz[:], ones16[:], brz_hi[:], start=False, stop=False)
        nc.tensor.matmul(ps_rz[:], ones16[:], brz_lo[:], start=False, stop=True)

        ps_hn = ps_sm.tile([128, 128], f32, tag="tp")
        for kc in range(UC):
            nc.tensor.matmul(
                ps_hn[:],
                hT_full[kc][:],
                wnhh[:, kc * US : (kc + 1) * US],
                start=(kc == 0),
                stop=False,
            )
        nc.tensor.matmul(ps_hn[:], ones16[:], bhn_hi[:], start=False, stop=False)
        nc.tensor.matmul(ps_hn[:], ones16[:], bhn_lo[:], start=False, stop=True)

        ps_in = ps_sm.tile([128, 128], f32, tag="tp")
        for j, kc in enumerate([4, 5, 6, 7, 0, 1, 2, 3]):
            nc.tensor.matmul(
                ps_in[:],
                ginT[:, kc * B : (kc + 1) * B],
                wnih[:, kc * US : (kc + 1) * US],
                start=(j == 0),
                stop=False,
            )
        nc.tensor.matmul(ps_in[:], ones16[:], bin_hi[:], start=False, stop=False)
        nc.tensor.matmul(ps_in[:], ones16[:], bin_lo[:], start=False, stop=True)

        r_sb = sb_act.tile([B, US], f32)
        nc.scalar.activation(r_sb[:], ps_rz[:, 0:US], AF.Sigmoid)
        z_sb = sb_act.tile([B, US], f32)
        nc.scalar.activation(z_sb[:], ps_rz[:, US : 2 * US], AF.Sigmoid)
        rhn = sb_act.tile([B, US], f32)
        nc.vector.tensor_tensor(out=rhn[:], in0=ps_hn[:], in1=r_sb[:], op=OP.mult)
        nc.vector.tensor_tensor(out=ps_in[:], in0=ps_in[:], in1=rhn[:], op=OP.add)
        n_sb = sb_act.tile([B, US], f32)
        nc.scalar.activation(n_sb[:], ps_in[:], AF.Tanh)
        hU = sb_act.tile([B, US], f32)
        nc.scalar.dma_start(hU[:], hU_d[:, :])
        d_sb = sb_act.tile([B, US], f32)
        nc.vector.tensor_tensor(out=d_sb[:], in0=hU[:], in1=n_sb[:], op=OP.subtract)
        zd_sb = sb_act.tile([B, US], f32)
        nc.vector.tensor_tensor(out=zd_sb[:], in0=z_sb[:], in1=d_sb[:], op=OP.mult)
        hnew_sb = sb_act.tile([B, US], f32)
        nc.vector.tensor_tensor(out=hnew_sb[:], in0=n_sb[:], in1=zd_sb[:], op=OP.add)
        nc.scalar.dma_start(hnew_d[:, :], hnew_sb[:])

        # ---------------- AllGather h_new (bf16) ----------------
        hnew16 = sb_act.tile([B, US], bf16)
        nc.vector.tensor_copy(hnew16[:], hnew_sb[:])
        hnew_bounce = dram.tile([B, US], bf16)
        hnew_ag = dram.tile([N_CORES * B, US], bf16)  # blocks [c][b, u_loc]
        nc.scalar.dma_start(hnew_bounce[:], hnew16[:])
        nc.gpsimd.collective_compute(
            "AllGather",
            OP.bypass,
            replica_groups=groups,
            ins=[hnew_bounce.opt()],
            outs=[hnew_ag.opt()],
        )

        # transpose blocks to h_new^T [u, b] chunks (bf16 for the fc matmul)
        hnT = sb_act.tile([128, UC * B], bf16)
        for c2 in range(N_CORES):
            blk = sb_tz.tile([128, US], bf16, tag="hn_blk")
            nc.scalar.dma_start(blk[:], hnew_ag[c2 * B : (c2 + 1) * B, :])
            pst = ps_sm.tile([128, 128], bf16, tag="tp", name="pst_hn")
            nc.tensor.matmul(pst[:], blk[:], ident16[:, :], is_transpose=True)
            nc.vector.tensor_copy(hnT[:, c2 * B : (c2 + 1) * B], pst[:])

        # ---------------- y = h_new @ fc_w^T (BN folded), replicated ----------------
        ybn_sb = sb_act.tile([B, U], f32)
        for half in range(2):
            ps_y = ps_big.tile([128, 512], f32, tag="big", name="ps_y")
            for kc in range(UC):
                nc.tensor.matmul(
                    ps_y[:],
                    hnT[:, kc * B : (kc + 1) * B],
                    fcw_wt[(half, kc)][:],
                    start=(kc == 0),
                    stop=(kc == UC - 1),
                )
            nc.vector.tensor_copy(ybn_sb[:, half * 512 : (half + 1) * 512], ps_y[:])

        fc2_in_dt = bf16 if FC2_BF16 else f32
        ybnT = sb_act.tile([128, UC * B], fc2_in_dt)
        for c2 in range(N_CORES):
            pst = ps_sm.tile([128, 128], f32, tag="tp")
            nc.tensor.matmul(
                pst[:], ybn_sb[:, c2 * US : (c2 + 1) * US], ident32[:, :],
                is_transpose=True,
            )
            nc.vector.tensor_copy(ybnT[:, c2 * B : (c2 + 1) * B], pst[:])

        # ---------------- fc2: logits = ybn @ fc2_w^T + fc2_b ----------------
        for vc in range(NVC):
            nv = min(512, VS - vc * 512)
            ps_l = ps_big.tile([128, 512], f32, tag="big")
            for kc in range(UC):
                wt = fc2_wt[(vc, kc)]
                nc.tensor.matmul(
                    ps_l[:, :nv],
                    ybnT[:, kc * B : (kc + 1) * B],
                    wt[:, :nv],
                    start=(kc == 0),
                    stop=False,
                )
            if FC2_BF16:
                nc.tensor.matmul(
                    ps_l[:, :nv],
                    ones16[:],
                    fc2bh[:, vc * 512 : vc * 512 + nv],
                    start=False,
                    stop=False,
                )
                nc.tensor.matmul(
                    ps_l[:, :nv],
                    ones16[:],
                    fc2bl[:, vc * 512 : vc * 512 + nv],
                    start=False,
                    stop=True,
                )
            else:
                nc.tensor.matmul(
                    ps_l[:, :nv],
                    ones32[:],
                    fc2b[:, vc * 512 : vc * 512 + nv],
                    start=False,
                    stop=True,
                )
            lg = sb_out.tile([B, 512], f32, tag="lg")
            nc.vector.tensor_copy(lg[:, :nv], ps_l[:, :nv])
            nc.scalar.dma_start(logits_d[:, vc * 512 : vc * 512 + nv], lg[:, :nv])

        ctxmgr.close()

    nc.compile()
    return nc


def _host_prep(inputs):
    import ml_dtypes

    bf16 = ml_dtypes.bfloat16
    f32 = np.float32

    x = np.asarray(inputs["x"])
    features = np.asarray(inputs["features"], dtype=f32)
    hidden = np.asarray(inputs["hidden"], dtype=f32)
    emb_table = np.asarray(inputs["emb_table"], dtype=f32)
    Uattn_w = np.asarray(inputs["Uattn_w"], dtype=f32)
    Uattn_b = np.asarray(inputs["Uattn_b"], dtype=f32)
    Wattn_w = np.asarray(inputs["Wattn_w"], dtype=f32)
    Wattn_b = np.asarray(inputs["Wattn_b"], dtype=f32)
    Vattn_w = np.asarray(inputs["Vattn_w"], dtype=f32)
    W_ih = np.asarray(inputs["W_ih"], dtype=f32)
    W_hh = np.asarray(inputs["W_hh"], dtype=f32)
    b_ih = np.asarray(inputs["b_ih"], dtype=f32)
    b_hh = np.asarray(inputs["b_hh"], dtype=f32)
    fc_w = np.asarray(inputs["fc_w"], dtype=f32)
    fc_b = np.asarray(inputs["fc_b"], dtype=f32)
    bn_gamma = np.asarray(inputs["bn_gamma"], dtype=f32)
    bn_beta = np.asarray(inputs["bn_beta"], dtype=f32)
    bn_mean = np.asarray(inputs["bn_mean"], dtype=f32)
    bn_var = np.asarray(inputs["bn_var"], dtype=f32)
    fc2_w = np.asarray(inputs["fc2_w"], dtype=f32)
    fc2_b = np.asarray(inputs["fc2_b"], dtype=f32)

    h = hidden[0]  # [B, U]
    hT16 = np.ascontiguousarray(h.T).astype(bf16)  # [U, B]
    uaT16 = np.ascontiguousarray(Uattn_w.T).astype(bf16)  # [EMB, U]
    waT16 = np.ascontiguousarray(Wattn_w.T).astype(bf16)  # [U, U]
    sbias = np.ascontiguousarray((Uattn_b + Wattn_b).reshape(UC, 128).T)  # [128, UC]
    vT16 = np.ascontiguousarray(Vattn_w[0].reshape(UC, 128).T).astype(bf16)
    e_full = emb_table[x[:, 0].astype(np.int64)]  # [B, EMB] row gather
    eT16 = np.ascontiguousarray(e_full.T).astype(bf16)  # [EMB, B]

    W_ihT = np.ascontiguousarray(W_ih.T)  # [U, 3U]
    W_hhT = np.ascontiguousarray(W_hh.T)
    bsum = (b_ih.astype(np.float64) + b_hh.astype(np.float64))  # [3U]

    bn_s = bn_gamma / np.sqrt(bn_var + BN_EPS)
    bn_t = bn_beta - bn_mean * bn_s
    fc_wTs16 = np.ascontiguousarray(fc_w.T * bn_s[None, :]).astype(bf16)  # [U, U]
    t2_full = (
        fc_b.astype(np.float64) * bn_s.astype(np.float64) + bn_t.astype(np.float64)
    )  # [U]

    fc2T = np.zeros((U, N_CORES * VS), dtype=f32)
    fc2T[:, :V] = fc2_w.T
    fc2b_pad = np.zeros(N_CORES * VS, dtype=np.float64)
    fc2b_pad[:V] = fc2_b.astype(np.float64) + t2_full @ fc2_w.T.astype(np.float64)

    def hilo(v):  # bf16 hi/lo split of a float64 vector -> [1, n] each
        hi = v.astype(bf16)
        lo = (v - hi.astype(np.float64)).astype(bf16)
        return hi.reshape(1, -1), lo.reshape(1, -1)

    in_maps = []
    for c in range(N_CORES):
        bc = slice(c * BC, (c + 1) * BC)
        Sc = slice(c * US, (c + 1) * US)
        feat_c = np.ascontiguousarray(features[bc].reshape(ROWS, EMB))
        m = {
            "feat16": feat_c.astype(bf16),
            "featT16": np.ascontiguousarray(feat_c.T).astype(bf16),
            "uaT16": uaT16,
            "waT16": waT16,
            "hT16": hT16,
            "hTbc16": np.ascontiguousarray(hT16[:, bc]),
            "hU": np.ascontiguousarray(h[:, Sc]),
            "sbias": sbias,
            "vT16": vT16,
            "eT16": eT16,
            "wrzih": np.ascontiguousarray(
                np.hstack([W_ihT[:, Sc], W_ihT[:, U + c * US : U + (c + 1) * US]])
            ).astype(bf16),
            "wrzhh": np.ascontiguousarray(
                np.hstack([W_hhT[:, Sc], W_hhT[:, U + c * US : U + (c + 1) * US]])
            ).astype(bf16),
            "wnih": np.ascontiguousarray(
                W_ihT[:, 2 * U + c * US : 2 * U + (c + 1) * US]
            ).astype(bf16),
            "wnhh": np.ascontiguousarray(
                W_hhT[:, 2 * U + c * US : 2 * U + (c + 1) * US]
            ).astype(bf16),
            "fcw": fc_wTs16,
            "fc2t": np.ascontiguousarray(fc2T[:, c * VS : (c + 1) * VS]).astype(
                bf16 if FC2_BF16 else f32
            ),
        }
        brz = np.concatenate([bsum[Sc], bsum[U + c * US : U + (c + 1) * US]])
        m["brz_hi"], m["brz_lo"] = hilo(brz)
        m["bin_hi"], m["bin_lo"] = hilo(
            b_ih[2 * U + c * US : 2 * U + (c + 1) * US].astype(np.float64)
        )
        m["bhn_hi"], m["bhn_lo"] = hilo(
            b_hh[2 * U + c * US : 2 * U + (c + 1) * US].astype(np.float64)
        )
        fb = fc2b_pad[c * VS : (c + 1) * VS]
        if FC2_BF16:
            m["fc2bh"], m["fc2bl"] = hilo(fb)
        else:
            m["fc2b"] = fb.astype(f32).reshape(1, VS)
        in_maps.append(m)
    return in_maps


def kernel(**inputs):
    from concourse.bass_utils import run_bass_kernel_spmd

    if "nc" not in _CACHE:
        _CACHE["nc"] = _build_bass()
    nc = _CACHE["nc"]
    in_maps = _host_prep(inputs)
    res = run_bass_kernel_spmd(nc, in_maps, list(range(N_CORES)))
    rs = res.results
    logits = np.concatenate([rs[c]["logits_c"] for c in range(N_CORES)], axis=1)[
        :, :V
    ]
    h_new = np.concatenate([rs[c]["hnew_c"] for c in range(N_CORES)], axis=1)[
        None, :, :
    ]
    attn = np.concatenate([rs[c]["attn_c"] for c in range(N_CORES)], axis=0)[
        :, :, None
    ]
    return logits.astype(np.float32), h_new.astype(np.float32), attn.astype(
        np.float32
    )
